# revision 24
# baseline (speedup 1.0000x reference)
"""Trainium2 Bass kernel for nn_Decoder (LSTM, B=128 T=512 H=1024 O=128).

Strategy: the T=512 recurrence is inherently sequential and one step's
recurrent matmul (h @ W_hh.T: 128x1024x4096) already saturates a single
NeuronCore's PE for ~9.5us, while any cross-core exchange of h costs an
8-core AllGather floor of ~5us + HBM bounces per step. Tensor-parallel
sharding therefore cannot beat replication, so every core runs the full
recurrence (weights and state replicated); the output is taken from core 0.

Per step (on each core):
  gates = [hT;x_t;1].T @ [W_hh.T; w_ih; b]   in bf16 on the PE,
          accumulated fp32 in PSUM, N=512 tiles, K tiled 8x128 (+K=2 aug).
  Gate columns are host-permuted per 128-wide H-chunk as [i|f|o|g] so one
  strided sigmoid covers i,f,o of a chunk pair and one tanh covers g.
  c (fp32) and h (bf16) updated on DVE; tanh/sigmoid on ACT;
  h chunks transposed back to lhsT layout [H,B] via the DMA xbar (2-byte).
"""

import os
import sys

sys.path.insert(0, "/opt/trn_rl_repo")
os.environ.setdefault("JAX_PLATFORMS", "")

from contextlib import ExitStack

import numpy as np
import ml_dtypes

import concourse.bass as bass
import concourse.mybir as mybir
import concourse.tile as tile
from concourse.bass import ds
from concourse.bass_utils import run_bass_kernel_spmd

B, T, H, O = 128, 512, 1024, 128
KC = H // 128          # 8 K-tiles of the contraction over H
NCH = H // 128         # 8 H-chunks of 128 hidden units
GW = 512               # gate columns per H-chunk: [i|f|o|g] x 128
BF16 = mybir.dt.bfloat16
F32 = mybir.dt.float32

_N_CORES = int(os.environ.get("KERNEL_N_CORES", "1"))
# steps per For_i body (must be even: hT ping-pong)
_UNROLL = int(os.environ.get("KERNEL_UNROLL", "4"))


# ---------------------------------------------------------------- drain patch
# walrus codegen limit: InstDrain on the SP engine accepts a single sync-wait
# command, but TileContext's exit drain aggregates one wait per outstanding
# logical processor onto one drain. Split them across a chain of drains.
def _apply_drain_patch():
    import concourse.tile as _tile
    from concourse.vector_clock import ScopedClock as _ScopedClock

    if getattr(_tile.TileContext, "_drain_patch_applied", False):
        return

    def _patched(self, tick_clock, wait_clock):
        drain_inst = self.nc.sync.drain()
        wait_clock.add_sem_waits(
            drain_inst.ins, _ScopedClock({None: tick_clock.global_clock})
        )
        si = drain_inst.ins.sync_info
        waits = list(si.on_wait) if si is not None and si.on_wait else []
        if len(waits) > 1:
            si.on_wait = waits[:1]
            for w in waits[1:]:
                extra = self.nc.sync.drain()
                extra.ins.sync_info = mybir.SyncInfo(on_wait=[w], on_update=[])
        self.nc.all_engine_barrier()
        assert self.sems is not None
        popped = self.nc._tile_sem_poison_stack.pop()
        assert popped is self._sem_poison
        self.nc.clear_and_free_semaphores(list(self.sems.allocated().values()))
        self.nc.all_engine_barrier()

    _tile.TileContext._drain_and_barrier = _patched
    _tile.TileContext._drain_patch_applied = True


# ----------------------------------------------------- wait-splitting post-pass
# This walrus build accepts at most 2 sync-wait commands on ordinary engine
# instructions and only 1 on SP/TPB_CTRL-class instructions (Drain, SP DMA
# triggers). Tile attaches up to ~4. Split the excess onto InstNoOp carriers
# inserted immediately before the offending instruction on the same engine.
_SP_LIKE = ("SP",)


def _wait_limit(inst):
    # empirically: TPB_CTRL (Drain) and S3S3D3_TT (TensorTensor) templates
    # accept a single sync-wait; play safe and allow one everywhere.
    return 1


def _split_excess_waits(nc):
    n_added = 0
    for f in nc.m.functions:
        for bb in f.blocks:
            insts = bb.instructions
            out = []
            changed = False
            for inst in insts:
                si = inst.sync_info
                waits = list(si.on_wait) if si is not None and si.on_wait else []
                lim = _wait_limit(inst)
                if len(waits) > lim:
                    keep = waits[len(waits) - lim :]
                    rest = waits[: len(waits) - lim]
                    nop_lim = 1
                    while rest:
                        chunk, rest = rest[:nop_lim], rest[nop_lim:]
                        nop = mybir.InstNoOp(
                            name=f"waitnop-{n_added}", ins=[], outs=[]
                        )
                        nop.engine = inst.engine
                        nop.sync_info = mybir.SyncInfo(on_wait=chunk, on_update=[])
                        out.append(nop)
                        n_added += 1
                    si.on_wait = keep
                    changed = True
                out.append(inst)
            if changed:
                bb.instructions = out
    return n_added


# ------------------------------------------------- ldweights dedup post-pass
# The IR legalization inserts one InstLdweights per InstMatmult, even when
# consecutive matmuls use the identical stationary AP. The PE array retains
# loaded weights across matmuls, so a reload of the exact same AP (with no
# intervening PE weight change and no write to that tensor) is redundant.
# Removing it saves ~50ns of serial PE time per load on HW.
def _dedup_ldweights(nc):
    n_removed = 0
    for f in nc.m.functions:
        for bb in f.blocks:
            out = []
            last_key = None
            last_tensor = None
            for inst in bb.instructions:
                tn = type(inst).__name__
                if tn == "InstLdweights":
                    key = str(inst.ins[0])
                    if key == last_key:
                        # redundant: drop, but carry sync info onto the
                        # next PE instruction (its matmult follows).
                        si = inst.sync_info
                        if si is not None and (si.on_wait or si.on_update):
                            carry = si
                        else:
                            carry = None
                        n_removed += 1
                        if carry is not None:
                            out.append(("carry", carry))
                        continue
                    last_key = key
                    last_tensor = _ap_tensor_name(inst.ins[0])
                else:
                    # any write to the currently-loaded tensor invalidates
                    if last_tensor is not None:
                        for o in inst.outs:
                            if _ap_tensor_name(o) == last_tensor:
                                last_key = None
                                last_tensor = None
                                break
                out.append(inst)
            # merge carried sync infos onto the next same-engine instruction
            merged = []
            pending = []
            for item in out:
                if isinstance(item, tuple):
                    pending.append(item[1])
                    continue
                if pending and getattr(item, "engine", None) == mybir.EngineType.PE:
                    si = item.sync_info
                    waits = list(si.on_wait) if si is not None and si.on_wait else []
                    updates = (
                        list(si.on_update) if si is not None and si.on_update else []
                    )
                    for c in pending:
                        if c.on_wait:
                            waits.extend(c.on_wait)
                        if c.on_update:
                            updates.extend(c.on_update)
                    item.sync_info = mybir.SyncInfo(on_wait=waits, on_update=updates)
                    pending = []
                merged.append(item)
            assert not pending
            bb.instructions = merged
    return n_removed


def _ap_tensor_name(arg):
    try:
        return arg.memory_location().name
    except Exception:
        try:
            return arg.tensor_name
        except Exception:
            return None


# ------------------------------------------------------------- program build
def build_program(t_steps=T, unroll=_UNROLL, debug_state=False, split_waits=True):
    _apply_drain_patch()
    assert t_steps % unroll == 0 and unroll % 2 == 0
    nc = bass.Bass("TRN2", debug=False)

    wt_d = nc.dram_tensor("wt", (H, 4 * H), BF16, kind="ExternalInput").ap()
    wihb_d = nc.dram_tensor("wihb", (2, 4 * H), BF16, kind="ExternalInput").ap()
    xa_d = nc.dram_tensor("xa", (2 * T, B), BF16, kind="ExternalInput").ap()
    wrep_d = nc.dram_tensor("wrep", (128, 4 * H), BF16, kind="ExternalInput").ap()
    brep_d = nc.dram_tensor("brep", (128, 4 * H), BF16, kind="ExternalInput").ap()
    xcol_d = nc.dram_tensor("xcol", (B, T), F32, kind="ExternalInput").ap()
    ht0_d = nc.dram_tensor("ht0", (H, B), BF16, kind="ExternalInput").ap()
    c0_d = nc.dram_tensor("c0", (B, H), F32, kind="ExternalInput").ap()
    fcw_d = nc.dram_tensor("fcw", (128, H), BF16, kind="ExternalInput").ap()
    fca_d = nc.dram_tensor("fca", (2, 128), BF16, kind="ExternalInput").ap()
    id_d = nc.dram_tensor("ident", (128, 128), BF16, kind="ExternalInput").ap()
    out_d = nc.dram_tensor("out", (B, O), F32, kind="ExternalOutput").ap()
    if debug_state:
        ht_dbg_d = nc.dram_tensor(
            "ht_dbg", (NCH, 128, B), BF16, kind="ExternalOutput"
        ).ap()
        c_dbg_d = nc.dram_tensor("c_dbg", (B, H), F32, kind="ExternalOutput").ap()

    with tile.TileContext(nc) as tc:
        with ExitStack() as ctx:
            consts = ctx.enter_context(tc.tile_pool(name="consts", bufs=1))
            state = ctx.enter_context(tc.tile_pool(name="state", bufs=1))
            work = ctx.enter_context(tc.tile_pool(name="work", bufs=int(os.environ.get("WORK_BUFS", "3"))))
            xap = ctx.enter_context(tc.tile_pool(name="xap", bufs=int(os.environ.get("XA_BUFS", "4"))))
            kouter = os.environ.get("KOUTER", "1") == "1"
            augmode = int(os.environ.get("AUGDVE", "0")) if kouter else 0
            augdve = augmode > 0
            if kouter:
                # 7 single-bank gate psums + 1 FC bank fill all of PSUM; the
                # final FC psum is drawn from the same pool after the loop.
                psum = ctx.enter_context(
                    tc.tile_pool(name="psum", bufs=7, space="PSUM")
                )
                fcp = psum
                ptp = None
            else:
                psum = ctx.enter_context(
                    tc.tile_pool(name="psum", bufs=3, space="PSUM")
                )
                fcp = ctx.enter_context(tc.tile_pool(name="fcp", bufs=1, space="PSUM"))
                ptp = ctx.enter_context(tc.tile_pool(name="ptp", bufs=1, space="PSUM"))


            # resident weights
            w_sb = []
            for k in range(KC):
                w_k = consts.tile([128, 4 * H], BF16, tag=f"w{k}", name=f"w{k}")
                nc.gpsimd.dma_start(out=w_k, in_=wt_d[k * 128 : (k + 1) * 128, :])
                w_sb.append(w_k)
            wihb = consts.tile([2, 4 * H], BF16, tag="wihb")
            nc.gpsimd.dma_start(out=wihb, in_=wihb_d)
            if augdve:
                wrep = consts.tile([128, 4 * H], BF16, tag="wrep")
                nc.gpsimd.dma_start(out=wrep, in_=wrep_d)
                brep = consts.tile([128, 4 * H], BF16, tag="brep")
                nc.gpsimd.dma_start(out=brep, in_=brep_d)
                xcp = ctx.enter_context(tc.tile_pool(name="xcp", bufs=4))
            fcw = consts.tile([128, H], BF16, tag="fcw")
            nc.gpsimd.dma_start(out=fcw, in_=fcw_d)
            ident = consts.tile([128, 128], BF16, tag="ident")
            nc.gpsimd.dma_start(out=ident, in_=id_d)
            fcb_t = consts.tile([1, 128], BF16, tag="fcb_t")
            nc.gpsimd.dma_start(out=fcb_t, in_=fca_d[0:1, :])
            ones_t = consts.tile([1, 128], BF16, tag="ones_t")
            nc.gpsimd.dma_start(out=ones_t, in_=fca_d[1:2, :])

            # state: hT ping-pong chunk tiles, fp32 cell
            ht_a = [state.tile([128, B], BF16, tag=f"hta{k}", name=f"hta{k}") for k in range(NCH)]
            ht_b = [state.tile([128, B], BF16, tag=f"htb{k}", name=f"htb{k}") for k in range(NCH)]
            c_sb = state.tile([B, H], F32, tag="c")
            for k in range(NCH):
                nc.gpsimd.dma_start(
                    out=ht_a[k], in_=ht0_d[k * 128 : (k + 1) * 128, :]
                )
            nc.gpsimd.dma_start(out=c_sb, in_=c0_d)

            def emit_step_kouter(iv_base, local_t, cur, nxt):
                """One LSTM step, k-outer: per half (4 gate-chunks), each
                stationary (h chunk / xa) is loaded once and streams all 4
                chunk columns; _dedup_ldweights removes the redundant
                reloads. Gate psums use 8 single-bank tiles; all h
                transposes go through the DMA xbar."""
                if not augdve:
                    xa = xap.tile([2, B], BF16, tag="xa")
                    if isinstance(iv_base, int):
                        off = 2 * (iv_base + local_t)
                        nc.sync.dma_start(out=xa, in_=xa_d[off : off + 2, :])
                    else:
                        off = (iv_base + local_t) * 2
                        nc.sync.dma_start(out=xa, in_=xa_d[ds(off, 2), :])

                def eltwise_single(cc, ps1):
                    sig1 = work.tile([B, 384], BF16, tag="sig1", name="sig1")
                    nc.scalar.activation(
                        sig1, ps1[:, 0:384], mybir.ActivationFunctionType.Sigmoid
                    )
                    tg1 = work.tile([B, 128], BF16, tag="tg1", name="tg1")
                    nc.scalar.activation(
                        tg1, ps1[:, 384:512], mybir.ActivationFunctionType.Tanh
                    )
                    c1 = c_sb[:, cc * 128 : (cc + 1) * 128]
                    t1s = work.tile([B, 128], F32, tag="t1s", name="t1s")
                    nc.vector.tensor_mul(t1s, sig1[:, 128:256], c1)
                    t2s = work.tile([B, 128], BF16, tag="t2s", name="t2s")
                    nc.vector.tensor_mul(t2s, sig1[:, 0:128], tg1)
                    nc.vector.tensor_add(c1, t1s, t2s)
                    tanc1 = work.tile([B, 128], BF16, tag="tanc1", name="tanc1")
                    nc.scalar.activation(
                        tanc1, c1, mybir.ActivationFunctionType.Tanh
                    )
                    hbf1 = work.tile([B, 128], BF16, tag="hbf1", name="hbf1")
                    nc.vector.tensor_mul(hbf1, sig1[:, 256:384], tanc1)
                    nc.sync.dma_start_transpose(nxt[cc], hbf1)

                if augdve:
                    xcol = xcp.tile([B, 1], F32, tag="xcol")
                    if isinstance(iv_base, int):
                        xo = iv_base + local_t
                        nc.sync.dma_start(out=xcol, in_=xcol_d[:, xo : xo + 1])
                    else:
                        nc.sync.dma_start(
                            out=xcol, in_=xcol_d[:, ds((iv_base + local_t) * 1, 1)]
                        )

                if augmode == 2:
                    # aug = x*w_ih + b computed in SBUF off the critical
                    # path; each chunk gets one DVE add after its matmuls.
                    aug_sb = work.tile([B, 4 * H], BF16, tag="aug", name="aug")
                    nc.vector.tensor_scalar(
                        aug_sb, wrep, xcol, None, mybir.AluOpType.mult
                    )
                    nc.vector.tensor_add(aug_sb, aug_sb, brep)

                for half in range(2):
                    ccs = [0, 1, 2, 3] if half == 0 else [4, 5, 6, 7]
                    pss = {
                        cc: psum.tile([B, GW], F32, tag="gates", name=f"ps{cc}")
                        for cc in ccs
                    }
                    if augmode == 1:
                        # pre-write x*w_ih + b into PSUM on the DVE; the k
                        # matmuls then accumulate on top (start=False).
                        for cc in ccs:
                            taug = work.tile([B, GW], BF16, tag="taug", name="taug")
                            nc.vector.tensor_scalar(
                                taug, wrep[:, cc * GW : (cc + 1) * GW],
                                xcol, None, mybir.AluOpType.mult,
                            )
                            nc.vector.tensor_add(
                                pss[cc], taug, brep[:, cc * GW : (cc + 1) * GW]
                            )
                    for k in range(KC):
                        for cc in ccs:
                            nc.tensor.matmul(
                                pss[cc], lhsT=cur[k],
                                rhs=w_sb[k][:, cc * GW : (cc + 1) * GW],
                                start=(k == 0 and augmode != 1),
                                stop=(augdve and k == KC - 1),
                                skip_group_check=augmode == 1,
                            )
                    if not augdve:
                        for cc in ccs:
                            nc.tensor.matmul(
                                pss[cc], lhsT=xa,
                                rhs=wihb[:, cc * GW : (cc + 1) * GW],
                                start=False, stop=True,
                            )
                    # chunks 6,7 are on the next step's critical path: run
                    # their aug-add + eltwise + transpose first in the half
                    order = ccs if half == 0 else [6, 7, 4, 5]
                    if augmode == 2:
                        for cc in order:
                            nc.vector.tensor_add(
                                pss[cc], pss[cc],
                                aug_sb[:, cc * GW : (cc + 1) * GW],
                            )
                    for cc in order:
                        eltwise_single(cc, pss[cc])

            def emit_step_piped(iv_base, local_t, cur, nxt, pending_in):
                """One LSTM step, software-pipelined across the step boundary.

                pending_in: closures (PE transposes + DVE copies of the
                PREVIOUS step's pair-3 h chunks) to emit after this step's
                first independent matmul block. Returns pending_out for the
                next step (empty when this is the last step of the body).
                """
                xa = xap.tile([2, B], BF16, tag="xa")
                if isinstance(iv_base, int):
                    off = 2 * (iv_base + local_t)
                    nc.sync.dma_start(out=xa, in_=xa_d[off : off + 2, :])
                else:
                    off = (iv_base + local_t) * 2
                    nc.sync.dma_start(out=xa, in_=xa_d[ds(off, 2), :])

                def mm(sl, k, cc, start, stop):
                    if k == "aug":
                        nc.tensor.matmul(
                            sl, lhsT=xa, rhs=wihb[:, cc * GW : (cc + 1) * GW],
                            start=start, stop=stop,
                        )
                    else:
                        nc.tensor.matmul(
                            sl, lhsT=cur[k],
                            rhs=w_sb[k][:, cc * GW : (cc + 1) * GW],
                            start=start, stop=stop,
                        )

                def eltwise(p, ps):
                    """gates [B, 2, 512] per chunk-pair -> hbf [B,2,128]."""
                    ps3 = ps.rearrange("p (c x) -> p c x", c=2)
                    sig = work.tile([B, 2, 384], BF16, tag="sig", name="sig")
                    nc.scalar.activation(
                        sig, ps3[:, :, 0:384],
                        mybir.ActivationFunctionType.Sigmoid,
                    )
                    tg = work.tile([B, 2, 128], BF16, tag="tg", name="tg")
                    nc.scalar.activation(
                        tg, ps3[:, :, 384:512], mybir.ActivationFunctionType.Tanh
                    )
                    c3 = c_sb[:, p * 256 : (p + 1) * 256].rearrange(
                        "p (c x) -> p c x", c=2
                    )
                    t1 = work.tile([B, 2, 128], F32, tag="t1", name="t1")
                    nc.vector.tensor_mul(t1, sig[:, :, 128:256], c3)
                    t2 = work.tile([B, 2, 128], BF16, tag="t2", name="t2")
                    nc.vector.tensor_mul(t2, sig[:, :, 0:128], tg)
                    nc.vector.tensor_add(c3, t1, t2)
                    tanc = work.tile([B, 2, 128], BF16, tag="tanc", name="tanc")
                    nc.scalar.activation(
                        tanc, c3, mybir.ActivationFunctionType.Tanh
                    )
                    hbf = work.tile([B, 2, 128], BF16, tag="hbf", name="hbf")
                    nc.vector.tensor_mul(hbf, sig[:, :, 256:384], tanc)
                    return hbf

                def pe_transpose_pair(p, hbf):
                    """PE-transpose both chunks of pair p into nxt (closures)."""
                    outs = []
                    for half in range(2):
                        def do(h=half):
                            pt = ptp.tile([128, B], BF16, tag="pt", name="pt")
                            nc.tensor.transpose(pt, hbf[:, h, :], ident)
                            nc.vector.tensor_copy(nxt[2 * p + h], pt)
                        outs.append(do)
                    return outs

                # ---- P0: k6,k7 deferred past the pending block ----
                ps0 = psum.tile([B, 2 * GW], F32, tag="gates", name="ps0")
                for half in range(2):
                    sl = ps0[:, half * GW : (half + 1) * GW]
                    for k in range(6):
                        mm(sl, k, half, start=(k == 0), stop=False)
                    mm(sl, "aug", half, start=False, stop=False)
                for fn in pending_in:
                    fn()
                for k in (6, 7):
                    for half in range(2):
                        mm(ps0[:, half * GW : (half + 1) * GW], k, half,
                           start=False, stop=(k == 7 and half == 1))
                hbf0 = eltwise(0, ps0)
                for half in range(2):
                    nc.sync.dma_start_transpose(nxt[half], hbf0[:, half, :])

                # ---- P1: standard order, DMA transposes ----
                ps1 = psum.tile([B, 2 * GW], F32, tag="gates", name="ps1")
                for half in range(2):
                    cc = 2 + half
                    sl = ps1[:, half * GW : (half + 1) * GW]
                    for k in range(6):
                        mm(sl, k, cc, start=(k == 0), stop=False)
                    mm(sl, "aug", cc, start=False, stop=False)
                    for k in (6, 7):
                        mm(sl, k, cc, start=False, stop=(k == 7))
                hbf1 = eltwise(1, ps1)
                for half in range(2):
                    nc.sync.dma_start_transpose(nxt[2 + half], hbf1[:, half, :])

                # ---- P2: PE transposes deferred into P3's MM stream ----
                ps2 = psum.tile([B, 2 * GW], F32, tag="gates", name="ps2")
                for half in range(2):
                    cc = 4 + half
                    sl = ps2[:, half * GW : (half + 1) * GW]
                    for k in range(6):
                        mm(sl, k, cc, start=(k == 0), stop=False)
                    mm(sl, "aug", cc, start=False, stop=False)
                    for k in (6, 7):
                        mm(sl, k, cc, start=False, stop=(k == 7))
                hbf2 = eltwise(2, ps2)
                t2_closures = pe_transpose_pair(2, hbf2)

                # ---- P3: first half interleaves P2's transposes ----
                ps3t = psum.tile([B, 2 * GW], F32, tag="gates", name="ps3")
                sl = ps3t[:, 0:GW]
                for k in range(6):
                    mm(sl, k, 6, start=(k == 0), stop=False)
                mm(sl, "aug", 6, start=False, stop=False)
                for fn in t2_closures:
                    fn()
                for k in (6, 7):
                    mm(sl, k, 6, start=False, stop=False)
                sl = ps3t[:, GW : 2 * GW]
                for k in range(6):
                    mm(sl, k, 7, start=(k == 0), stop=False)
                mm(sl, "aug", 7, start=False, stop=False)
                for k in (6, 7):
                    mm(sl, k, 7, start=False, stop=(k == 7))
                hbf3 = eltwise(3, ps3t)
                return pe_transpose_pair(3, hbf3)

            def step(iv_base, local_t, cur, nxt):
                """One LSTM step. iv_base: ScalarValue or int (loop index of the
                body start); local_t: python int offset within the body."""
                xa = xap.tile([2, B], BF16, tag="xa")
                # inside the For_i body only HWDGE DMAs are usable: the loop
                # reset block emits InstIncSwdgeSem for SWDGE queues, which
                # this walrus cannot encode ("ISA wrong length").
                if isinstance(iv_base, int):
                    off = 2 * (iv_base + local_t)
                    nc.sync.dma_start(out=xa, in_=xa_d[off : off + 2, :])
                else:
                    off = (iv_base + local_t) * 2
                    nc.sync.dma_start(out=xa, in_=xa_d[ds(off, 2), :])

                n_pairs = 3 if os.environ.get("TAIL_SINGLE", "0") == "1" else 4
                for p in range(n_pairs):  # pairs of H-chunks
                    ps = psum.tile([B, 2 * GW], F32, tag="gates", name=f"ps{p}")
                    for half in range(2):
                        cc = 2 * p + half
                        sl = ps[:, half * GW : (half + 1) * GW]
                        # K-order [0..5, aug, 6, 7]: defers the previous
                        # step's latest h-chunks by two MM slots, shrinking
                        # the step-boundary stall. Same PSUM group, so no
                        # tile-switch penalty.
                        for k in range(6):
                            nc.tensor.matmul(
                                sl,
                                lhsT=cur[k],
                                rhs=w_sb[k][:, cc * GW : (cc + 1) * GW],
                                start=(k == 0),
                                stop=False,
                            )
                        nc.tensor.matmul(
                            sl,
                            lhsT=xa,
                            rhs=wihb[:, cc * GW : (cc + 1) * GW],
                            start=False,
                            stop=False,
                        )
                        for k in (6, 7):
                            nc.tensor.matmul(
                                sl,
                                lhsT=cur[k],
                                rhs=w_sb[k][:, cc * GW : (cc + 1) * GW],
                                start=False,
                                stop=(k == KC - 1),
                            )
                    # eltwise; psum layout [i0 f0 o0 g0 i1 f1 o1 g1]
                    ps3 = ps.rearrange("p (c x) -> p c x", c=2)
                    sig = work.tile([B, 2, 384], BF16, tag="sig", name="sig")
                    nc.scalar.activation(
                        sig, ps3[:, :, 0:384], mybir.ActivationFunctionType.Sigmoid
                    )
                    tg = work.tile([B, 2, 128], BF16, tag="tg", name="tg")
                    nc.scalar.activation(
                        tg, ps3[:, :, 384:512], mybir.ActivationFunctionType.Tanh
                    )
                    sig_i = sig[:, :, 0:128]
                    sig_f = sig[:, :, 128:256]
                    sig_o = sig[:, :, 256:384]
                    c3 = c_sb[:, p * 256 : (p + 1) * 256].rearrange(
                        "p (c x) -> p c x", c=2
                    )
                    t1 = work.tile([B, 2, 128], F32, tag="t1", name="t1")
                    nc.vector.tensor_mul(t1, sig_f, c3)
                    t2 = work.tile([B, 2, 128], BF16, tag="t2", name="t2")
                    nc.vector.tensor_mul(t2, sig_i, tg)
                    nc.vector.tensor_add(c3, t1, t2)
                    tanc = work.tile([B, 2, 128], BF16, tag="tanc", name="tanc")
                    nc.scalar.activation(
                        tanc, c3, mybir.ActivationFunctionType.Tanh
                    )
                    hbf = work.tile([B, 2, 128], BF16, tag="hbf", name="hbf")
                    nc.vector.tensor_mul(hbf, sig_o, tanc)
                    for half in range(2):
                        if p >= 2:
                            # last pair is on the next step's critical path:
                            # PE transpose (~0.4us) beats the DMA xbar (~1.3us)
                            pt = ptp.tile([128, B], BF16, tag="pt", name="pt")
                            nc.tensor.transpose(pt, hbf[:, half, :], ident)
                            nc.vector.tensor_copy(nxt[2 * p + half], pt)
                        else:
                            nc.sync.dma_start_transpose(
                                nxt[2 * p + half], hbf[:, half, :]
                            )

                for cc in range(2 * n_pairs, NCH):  # tail chunks, single width
                    ps1 = psum.tile([B, GW], F32, tag="gates", name=f"ps1_{cc}")
                    for k in range(KC):
                        nc.tensor.matmul(
                            ps1, lhsT=cur[k],
                            rhs=w_sb[k][:, cc * GW : (cc + 1) * GW],
                            start=(k == 0), stop=False,
                        )
                    nc.tensor.matmul(
                        ps1, lhsT=xa, rhs=wihb[:, cc * GW : (cc + 1) * GW],
                        start=False, stop=True,
                    )
                    sig1 = work.tile([B, 384], BF16, tag="sig1", name="sig1")
                    nc.scalar.activation(
                        sig1, ps1[:, 0:384], mybir.ActivationFunctionType.Sigmoid
                    )
                    tg1 = work.tile([B, 128], BF16, tag="tg1", name="tg1")
                    nc.scalar.activation(
                        tg1, ps1[:, 384:512], mybir.ActivationFunctionType.Tanh
                    )
                    c1 = c_sb[:, cc * 128 : (cc + 1) * 128]
                    t1s = work.tile([B, 128], F32, tag="t1s", name="t1s")
                    nc.vector.tensor_mul(t1s, sig1[:, 128:256], c1)
                    t2s = work.tile([B, 128], BF16, tag="t2s", name="t2s")
                    nc.vector.tensor_mul(t2s, sig1[:, 0:128], tg1)
                    nc.vector.tensor_add(c1, t1s, t2s)
                    tanc1 = work.tile([B, 128], BF16, tag="tanc1", name="tanc1")
                    nc.scalar.activation(
                        tanc1, c1, mybir.ActivationFunctionType.Tanh
                    )
                    hbf1 = work.tile([B, 128], BF16, tag="hbf1", name="hbf1")
                    nc.vector.tensor_mul(hbf1, sig1[:, 256:384], tanc1)
                    pt1 = ptp.tile([128, B], BF16, tag="pt", name="pt1")
                    nc.tensor.transpose(pt1, hbf1, ident)
                    nc.vector.tensor_copy(nxt[cc], pt1)

            pipe = os.environ.get("PIPE", "0") == "1"

            def emit_body(iv_base, n_steps):
                pending = []
                for j in range(n_steps):
                    cur, nxt = (ht_a, ht_b) if j % 2 == 0 else (ht_b, ht_a)
                    if kouter:
                        emit_step_kouter(iv_base, j, cur, nxt)
                    elif pipe:
                        pending = emit_step_piped(iv_base, j, cur, nxt, pending)
                    else:
                        step(iv_base, j, cur, nxt)
                for fn in pending:  # flush at body boundary
                    fn()

            if t_steps == 0:
                pass
            elif t_steps <= unroll:
                repeat_u = int(os.environ.get("KERNEL_REPEAT", "1"))

                if repeat_u == 1:
                    emit_body(0, t_steps)
                else:
                    with tc.For_i(0, repeat_u, 1):
                        emit_body(0, t_steps)
            else:
                hints = tuple(mybir.ALL_ENGINES) if os.environ.get("HINTS", "0") == "1" else ()
                repeat = int(os.environ.get("KERNEL_REPEAT", "1"))

                def inner_loop():
                    with tc.For_i(0, t_steps, unroll, hint_engines=hints) as iv:
                        emit_body(iv, unroll)

                if repeat == 1:
                    inner_loop()
                else:  # timing amplification only: state re-evolves from t=0 xs
                    with tc.For_i(0, repeat, 1):
                        inner_loop()

            ht_fin = ht_a if t_steps % 2 == 0 else ht_b

            # final FC: out = h_T @ fc_W.T + fc_b
            fc_ps = fcp.tile([B, O], F32, tag="fc", name="fcps", bufs=1)
            nc.tensor.matmul(
                fc_ps, lhsT=ones_t, rhs=fcb_t, start=True, stop=False
            )
            for k in range(KC):
                nc.tensor.matmul(
                    fc_ps,
                    lhsT=ht_fin[k],
                    rhs=fcw[:, k * 128 : (k + 1) * 128],
                    start=False,
                    stop=(k == KC - 1),
                )
            out_sb = work.tile([B, O], F32, tag="out_sb")
            nc.vector.tensor_copy(out_sb, fc_ps)
            nc.gpsimd.dma_start(out=out_d, in_=out_sb)

            if debug_state:
                for k in range(NCH):
                    nc.gpsimd.dma_start(out=ht_dbg_d[k], in_=ht_fin[k])
                nc.gpsimd.dma_start(out=c_dbg_d, in_=c_sb)

    if os.environ.get("DEDUP", "1") == "1":
        _dedup_ldweights(nc)
    if split_waits:  # required for walrus codegen; CoreSim chokes on it
        _split_excess_waits(nc)
    return nc


# ------------------------------------------------------------------ host prep
def _prep_inputs(y_hist, W_ih, W_hh, b_ih, b_hh, fc_W, fc_b, h0, c0):
    f32 = np.float32
    bf16 = ml_dtypes.bfloat16
    # per-chunk gate permutation of the 4H rows: [i_c | f_c | o_c | g_c]
    # reference gate order in rows of W_hh is (i, f, g, o) blocks of H
    perm = np.concatenate(
        [
            g * H + c * 128 + np.arange(128)
            for c in range(NCH)
            for g in (0, 1, 3, 2)
        ]
    )
    wt = np.ascontiguousarray(W_hh[perm, :].T).astype(bf16)          # (H, 4H)
    wihb = np.stack([W_ih[:, 0][perm], (b_ih + b_hh)[perm]]).astype(bf16)
    xa = np.empty((2 * T, B), f32)
    xa[0::2] = y_hist.T                                               # x_t rows
    xa[1::2] = 1.0                                                    # ones rows
    xa = xa.astype(bf16)
    ht0 = np.ascontiguousarray(h0.T).astype(bf16)                     # (H, B)
    fcw = np.ascontiguousarray(fc_W.T).astype(bf16)                  # (H, O)
    # device layout for fcw tile: (128, H) with chunk k at cols [128k:128k+128)
    fcw_tile = fcw.reshape(KC, 128, O).transpose(1, 0, 2).reshape(128, H)
    fca = np.stack([fc_b, np.ones(O, f32)]).astype(bf16)              # rhs, ones
    ident = np.eye(128, dtype=f32).astype(bf16)
    wrep = np.broadcast_to(W_ih[:, 0][perm][None, :], (128, 4 * H)).astype(bf16)
    brep = np.broadcast_to((b_ih + b_hh)[perm][None, :], (128, 4 * H)).astype(bf16)
    return {
        "ident": np.asarray(ident),
        "wt": np.asarray(wt),
        "wihb": np.asarray(wihb),
        "xa": np.asarray(xa),
        "wrep": np.ascontiguousarray(wrep),
        "brep": np.ascontiguousarray(brep),
        "xcol": np.ascontiguousarray(y_hist.astype(f32)),
        "ht0": np.asarray(ht0),
        "c0": c0.astype(f32),
        "fcw": np.asarray(fcw_tile),
        "fca": np.asarray(fca),
    }


_CACHE = {}


def _make_runner(nc):
    """Single-core reusable jitted executor (mirrors bass2jax.run_bass_via_pjrt
    but caches the jitted body so repeated kernel() calls skip retracing)."""
    import jax
    from concourse import bass2jax

    bass2jax.install_neuronx_cc_hook()
    partition_name = nc.partition_id_tensor.name if nc.partition_id_tensor else None
    in_names, out_names, out_avals, zero_outs = [], [], [], []
    for alloc in nc.m.functions[0].allocations:
        if not isinstance(alloc, mybir.MemoryLocationSet):
            continue
        name = alloc.memorylocations[0].name
        if alloc.kind == "ExternalInput":
            if name != partition_name:
                in_names.append(name)
        elif alloc.kind == "ExternalOutput":
            shape = tuple(alloc.tensor_shape)
            dtype = mybir.dt.np(alloc.dtype)
            out_names.append(name)
            out_avals.append(jax.core.ShapedArray(shape, dtype))
            zero_outs.append(np.zeros(shape, dtype))
    all_in = list(in_names) + list(out_names)
    if partition_name is not None:
        all_in.append(partition_name)

    def _body(*args):
        operands = list(args)
        if partition_name is not None:
            operands.append(bass2jax.partition_id_tensor())
        return tuple(
            bass2jax._bass_exec_p.bind(
                *operands,
                out_avals=tuple(out_avals),
                in_names=tuple(all_in),
                out_names=tuple(out_names),
                lowering_input_output_aliases=(),
                sim_require_finite=True,
                sim_require_nnan=True,
                nc=nc,
            )
        )

    f = jax.jit(_body, keep_unused=True)
    return f, in_names, out_names, zero_outs


def kernel(y_hist, W_ih, W_hh, b_ih, b_hh, fc_W, fc_b, h0, c0, **kw):
    dev_in = _prep_inputs(
        np.asarray(y_hist, np.float32),
        np.asarray(W_ih, np.float32),
        np.asarray(W_hh, np.float32),
        np.asarray(b_ih, np.float32),
        np.asarray(b_hh, np.float32),
        np.asarray(fc_W, np.float32),
        np.asarray(fc_b, np.float32),
        np.asarray(h0, np.float32),
        np.asarray(c0, np.float32),
    )
    if _N_CORES != 1:
        if "nc" not in _CACHE:
            _CACHE["nc"] = build_program()
        res = run_bass_kernel_spmd(
            _CACHE["nc"],
            [dict(dev_in) for _ in range(_N_CORES)],
            core_ids=list(range(_N_CORES)),
        )
        return np.asarray(res.results[0]["out"], np.float32)
    if "runner" not in _CACHE:
        nc = build_program()
        _CACHE["runner"] = _make_runner(nc)
    f, in_names, out_names, zero_outs = _CACHE["runner"]
    args = [np.asarray(dev_in[n]) for n in in_names] + zero_outs
    outs = f(*args)
    res = {n: np.asarray(outs[i]) for i, n in enumerate(out_names)}
    return np.asarray(res["out"], np.float32)



# revision 25
# speedup vs baseline: 1.0559x; 1.0559x over previous
"""Trainium2 Bass kernel for nn_Decoder (LSTM, B=128 T=512 H=1024 O=128).

Strategy: the T=512 recurrence is inherently sequential and one step's
recurrent matmul (h @ W_hh.T: 128x1024x4096) already saturates a single
NeuronCore's PE for ~9.5us, while any cross-core exchange of h costs an
8-core AllGather floor of ~5us + HBM bounces per step. Tensor-parallel
sharding therefore cannot beat replication, so every core runs the full
recurrence (weights and state replicated); the output is taken from core 0.

Per step (on each core), k-outer schedule (KOUTER=1 default):
  gates = [hT;x_t;1].T @ [W_hh.T; w_ih; b]   in bf16 on the PE,
          accumulated fp32 in PSUM. The 4096 gate columns are split in two
          halves of 4 single-bank [B,512] psums; within a half each
          stationary (h chunk k / xa) is loaded once and streams all 4
          chunk columns (k-outer), and _dedup_ldweights removes the
          redundant ldweights the legalizer would re-emit per matmul.
  Gate columns are host-permuted per 128-wide H-chunk as [i|f|o|g] so one
  strided sigmoid covers i,f,o of a chunk and one tanh covers g.
  c (fp32) and h (bf16) updated on DVE; tanh/sigmoid on ACT; h chunks
  transposed back to lhsT layout [H,B] via the DMA xbar, with chunks 6,7
  (needed earliest next step) processed first in the second half.
"""

import os
import sys

sys.path.insert(0, "/opt/trn_rl_repo")
os.environ.setdefault("JAX_PLATFORMS", "")

from contextlib import ExitStack

import numpy as np
import ml_dtypes

import concourse.bass as bass
import concourse.mybir as mybir
import concourse.tile as tile
from concourse.bass import ds
from concourse.bass_utils import run_bass_kernel_spmd

B, T, H, O = 128, 512, 1024, 128
KC = H // 128          # 8 K-tiles of the contraction over H
NCH = H // 128         # 8 H-chunks of 128 hidden units
GW = 512               # gate columns per H-chunk: [i|f|o|g] x 128
BF16 = mybir.dt.bfloat16
F32 = mybir.dt.float32

_N_CORES = int(os.environ.get("KERNEL_N_CORES", "1"))
# steps per For_i body (must be even: hT ping-pong)
_UNROLL = int(os.environ.get("KERNEL_UNROLL", "4"))


# ---------------------------------------------------------------- drain patch
# walrus codegen limit: InstDrain on the SP engine accepts a single sync-wait
# command, but TileContext's exit drain aggregates one wait per outstanding
# logical processor onto one drain. Split them across a chain of drains.
def _apply_drain_patch():
    import concourse.tile as _tile
    from concourse.vector_clock import ScopedClock as _ScopedClock

    if getattr(_tile.TileContext, "_drain_patch_applied", False):
        return

    def _patched(self, tick_clock, wait_clock):
        drain_inst = self.nc.sync.drain()
        wait_clock.add_sem_waits(
            drain_inst.ins, _ScopedClock({None: tick_clock.global_clock})
        )
        si = drain_inst.ins.sync_info
        waits = list(si.on_wait) if si is not None and si.on_wait else []
        if len(waits) > 1:
            si.on_wait = waits[:1]
            for w in waits[1:]:
                extra = self.nc.sync.drain()
                extra.ins.sync_info = mybir.SyncInfo(on_wait=[w], on_update=[])
        self.nc.all_engine_barrier()
        assert self.sems is not None
        popped = self.nc._tile_sem_poison_stack.pop()
        assert popped is self._sem_poison
        self.nc.clear_and_free_semaphores(list(self.sems.allocated().values()))
        self.nc.all_engine_barrier()

    _tile.TileContext._drain_and_barrier = _patched
    _tile.TileContext._drain_patch_applied = True


# ----------------------------------------------------- wait-splitting post-pass
# This walrus build accepts at most 2 sync-wait commands on ordinary engine
# instructions and only 1 on SP/TPB_CTRL-class instructions (Drain, SP DMA
# triggers). Tile attaches up to ~4. Split the excess onto InstNoOp carriers
# inserted immediately before the offending instruction on the same engine.
_SP_LIKE = ("SP",)


def _wait_limit(inst):
    # empirically: TPB_CTRL (Drain) and S3S3D3_TT (TensorTensor) templates
    # accept a single sync-wait; play safe and allow one everywhere.
    return 1


def _split_excess_waits(nc):
    n_added = 0
    for f in nc.m.functions:
        for bb in f.blocks:
            insts = bb.instructions
            out = []
            changed = False
            for inst in insts:
                si = inst.sync_info
                waits = list(si.on_wait) if si is not None and si.on_wait else []
                lim = _wait_limit(inst)
                if len(waits) > lim:
                    keep = waits[len(waits) - lim :]
                    rest = waits[: len(waits) - lim]
                    nop_lim = 1
                    while rest:
                        chunk, rest = rest[:nop_lim], rest[nop_lim:]
                        nop = mybir.InstNoOp(
                            name=f"waitnop-{n_added}", ins=[], outs=[]
                        )
                        nop.engine = inst.engine
                        nop.sync_info = mybir.SyncInfo(on_wait=chunk, on_update=[])
                        out.append(nop)
                        n_added += 1
                    si.on_wait = keep
                    changed = True
                out.append(inst)
            if changed:
                bb.instructions = out
    return n_added


# ------------------------------------------------- ldweights dedup post-pass
# The IR legalization inserts one InstLdweights per InstMatmult, even when
# consecutive matmuls use the identical stationary AP. The PE array retains
# loaded weights across matmuls, so a reload of the exact same AP (with no
# intervening PE weight change and no write to that tensor) is redundant.
# Removing it saves ~50ns of serial PE time per load on HW.
def _dedup_ldweights(nc):
    n_removed = 0
    for f in nc.m.functions:
        for bb in f.blocks:
            out = []
            last_key = None
            last_tensor = None
            for inst in bb.instructions:
                tn = type(inst).__name__
                if tn == "InstLdweights":
                    key = str(inst.ins[0])
                    if key == last_key:
                        # redundant: drop, but carry sync info onto the
                        # next PE instruction (its matmult follows).
                        si = inst.sync_info
                        if si is not None and (si.on_wait or si.on_update):
                            carry = si
                        else:
                            carry = None
                        n_removed += 1
                        if carry is not None:
                            out.append(("carry", carry))
                        continue
                    last_key = key
                    last_tensor = _ap_tensor_name(inst.ins[0])
                else:
                    # any write to the currently-loaded tensor invalidates
                    if last_tensor is not None:
                        for o in inst.outs:
                            if _ap_tensor_name(o) == last_tensor:
                                last_key = None
                                last_tensor = None
                                break
                out.append(inst)
            # merge carried sync infos onto the next same-engine instruction
            merged = []
            pending = []
            for item in out:
                if isinstance(item, tuple):
                    pending.append(item[1])
                    continue
                if pending and getattr(item, "engine", None) == mybir.EngineType.PE:
                    si = item.sync_info
                    waits = list(si.on_wait) if si is not None and si.on_wait else []
                    updates = (
                        list(si.on_update) if si is not None and si.on_update else []
                    )
                    for c in pending:
                        if c.on_wait:
                            waits.extend(c.on_wait)
                        if c.on_update:
                            updates.extend(c.on_update)
                    item.sync_info = mybir.SyncInfo(on_wait=waits, on_update=updates)
                    pending = []
                merged.append(item)
            assert not pending
            bb.instructions = merged
    return n_removed


def _ap_tensor_name(arg):
    try:
        return arg.memory_location().name
    except Exception:
        try:
            return arg.tensor_name
        except Exception:
            return None


# ------------------------------------------------------------- program build
def build_program(t_steps=T, unroll=_UNROLL, debug_state=False, split_waits=True):
    _apply_drain_patch()
    assert t_steps % unroll == 0 and unroll % 2 == 0
    nc = bass.Bass("TRN2", debug=False)

    wt_d = nc.dram_tensor("wt", (H, 4 * H), BF16, kind="ExternalInput").ap()
    wihb_d = nc.dram_tensor("wihb", (2, 4 * H), BF16, kind="ExternalInput").ap()
    xa_d = nc.dram_tensor("xa", (2 * T, B), BF16, kind="ExternalInput").ap()
    wrep_d = nc.dram_tensor("wrep", (128, 4 * H), BF16, kind="ExternalInput").ap()
    brep_d = nc.dram_tensor("brep", (128, 4 * H), BF16, kind="ExternalInput").ap()
    xcol_d = nc.dram_tensor("xcol", (B, T), F32, kind="ExternalInput").ap()
    ht0_d = nc.dram_tensor("ht0", (H, B), BF16, kind="ExternalInput").ap()
    c0_d = nc.dram_tensor("c0", (B, H), F32, kind="ExternalInput").ap()
    fcw_d = nc.dram_tensor("fcw", (128, H), BF16, kind="ExternalInput").ap()
    fca_d = nc.dram_tensor("fca", (2, 128), BF16, kind="ExternalInput").ap()
    id_d = nc.dram_tensor("ident", (128, 128), BF16, kind="ExternalInput").ap()
    out_d = nc.dram_tensor("out", (B, O), F32, kind="ExternalOutput").ap()
    if debug_state:
        ht_dbg_d = nc.dram_tensor(
            "ht_dbg", (NCH, 128, B), BF16, kind="ExternalOutput"
        ).ap()
        c_dbg_d = nc.dram_tensor("c_dbg", (B, H), F32, kind="ExternalOutput").ap()

    with tile.TileContext(nc) as tc:
        with ExitStack() as ctx:
            consts = ctx.enter_context(tc.tile_pool(name="consts", bufs=1))
            state = ctx.enter_context(tc.tile_pool(name="state", bufs=1))
            work = ctx.enter_context(tc.tile_pool(name="work", bufs=int(os.environ.get("WORK_BUFS", "3"))))
            xap = ctx.enter_context(tc.tile_pool(name="xap", bufs=int(os.environ.get("XA_BUFS", "4"))))
            kouter = os.environ.get("KOUTER", "1") == "1"
            augmode = int(os.environ.get("AUGDVE", "0")) if kouter else 0
            augdve = augmode > 0
            if kouter:
                # 7 single-bank gate psums + 1 FC bank fill all of PSUM; the
                # final FC psum is drawn from the same pool after the loop.
                psum = ctx.enter_context(
                    tc.tile_pool(name="psum", bufs=7, space="PSUM")
                )
                fcp = psum
                ptp = None
            else:
                psum = ctx.enter_context(
                    tc.tile_pool(name="psum", bufs=3, space="PSUM")
                )
                fcp = ctx.enter_context(tc.tile_pool(name="fcp", bufs=1, space="PSUM"))
                ptp = ctx.enter_context(tc.tile_pool(name="ptp", bufs=1, space="PSUM"))


            # resident weights
            w_sb = []
            for k in range(KC):
                w_k = consts.tile([128, 4 * H], BF16, tag=f"w{k}", name=f"w{k}")
                nc.gpsimd.dma_start(out=w_k, in_=wt_d[k * 128 : (k + 1) * 128, :])
                w_sb.append(w_k)
            wihb = consts.tile([2, 4 * H], BF16, tag="wihb")
            nc.gpsimd.dma_start(out=wihb, in_=wihb_d)
            if augdve:
                wrep = consts.tile([128, 4 * H], BF16, tag="wrep")
                nc.gpsimd.dma_start(out=wrep, in_=wrep_d)
                brep = consts.tile([128, 4 * H], BF16, tag="brep")
                nc.gpsimd.dma_start(out=brep, in_=brep_d)
                xcp = ctx.enter_context(tc.tile_pool(name="xcp", bufs=4))
            fcw = consts.tile([128, H], BF16, tag="fcw")
            nc.gpsimd.dma_start(out=fcw, in_=fcw_d)
            ident = consts.tile([128, 128], BF16, tag="ident")
            nc.gpsimd.dma_start(out=ident, in_=id_d)
            fcb_t = consts.tile([1, 128], BF16, tag="fcb_t")
            nc.gpsimd.dma_start(out=fcb_t, in_=fca_d[0:1, :])
            ones_t = consts.tile([1, 128], BF16, tag="ones_t")
            nc.gpsimd.dma_start(out=ones_t, in_=fca_d[1:2, :])

            # state: hT ping-pong chunk tiles, fp32 cell
            ht_a = [state.tile([128, B], BF16, tag=f"hta{k}", name=f"hta{k}") for k in range(NCH)]
            ht_b = [state.tile([128, B], BF16, tag=f"htb{k}", name=f"htb{k}") for k in range(NCH)]
            c_sb = state.tile([B, H], F32, tag="c")
            for k in range(NCH):
                nc.gpsimd.dma_start(
                    out=ht_a[k], in_=ht0_d[k * 128 : (k + 1) * 128, :]
                )
            nc.gpsimd.dma_start(out=c_sb, in_=c0_d)

            def emit_step_kouter(iv_base, local_t, cur, nxt):
                """One LSTM step, k-outer: per half (4 gate-chunks), each
                stationary (h chunk / xa) is loaded once and streams all 4
                chunk columns; _dedup_ldweights removes the redundant
                reloads. Gate psums use 8 single-bank tiles; all h
                transposes go through the DMA xbar."""
                if not augdve:
                    xa = xap.tile([2, B], BF16, tag="xa")
                    if isinstance(iv_base, int):
                        off = 2 * (iv_base + local_t)
                        nc.sync.dma_start(out=xa, in_=xa_d[off : off + 2, :])
                    else:
                        off = (iv_base + local_t) * 2
                        nc.sync.dma_start(out=xa, in_=xa_d[ds(off, 2), :])

                def eltwise_single(cc, ps1):
                    sig1 = work.tile([B, 384], BF16, tag="sig1", name="sig1")
                    nc.scalar.activation(
                        sig1, ps1[:, 0:384], mybir.ActivationFunctionType.Sigmoid
                    )
                    tg1 = work.tile([B, 128], BF16, tag="tg1", name="tg1")
                    nc.scalar.activation(
                        tg1, ps1[:, 384:512], mybir.ActivationFunctionType.Tanh
                    )
                    c1 = c_sb[:, cc * 128 : (cc + 1) * 128]
                    t1s = work.tile([B, 128], F32, tag="t1s", name="t1s")
                    nc.vector.tensor_mul(t1s, sig1[:, 128:256], c1)
                    t2s = work.tile([B, 128], BF16, tag="t2s", name="t2s")
                    nc.vector.tensor_mul(t2s, sig1[:, 0:128], tg1)
                    nc.vector.tensor_add(c1, t1s, t2s)
                    tanc1 = work.tile([B, 128], BF16, tag="tanc1", name="tanc1")
                    nc.scalar.activation(
                        tanc1, c1, mybir.ActivationFunctionType.Tanh
                    )
                    hbf1 = work.tile([B, 128], BF16, tag="hbf1", name="hbf1")
                    nc.vector.tensor_mul(hbf1, sig1[:, 256:384], tanc1)
                    nc.sync.dma_start_transpose(nxt[cc], hbf1)

                if augdve:
                    xcol = xcp.tile([B, 1], F32, tag="xcol")
                    if isinstance(iv_base, int):
                        xo = iv_base + local_t
                        nc.sync.dma_start(out=xcol, in_=xcol_d[:, xo : xo + 1])
                    else:
                        nc.sync.dma_start(
                            out=xcol, in_=xcol_d[:, ds((iv_base + local_t) * 1, 1)]
                        )

                if augmode == 2:
                    # aug = x*w_ih + b computed in SBUF off the critical
                    # path; each chunk gets one DVE add after its matmuls.
                    aug_sb = work.tile([B, 4 * H], BF16, tag="aug", name="aug")
                    nc.vector.tensor_scalar(
                        aug_sb, wrep, xcol, None, mybir.AluOpType.mult
                    )
                    nc.vector.tensor_add(aug_sb, aug_sb, brep)

                for half in range(2):
                    ccs = [0, 1, 2, 3] if half == 0 else [4, 5, 6, 7]
                    pss = {
                        cc: psum.tile([B, GW], F32, tag="gates", name=f"ps{cc}")
                        for cc in ccs
                    }
                    if augmode == 1:
                        # pre-write x*w_ih + b into PSUM on the DVE; the k
                        # matmuls then accumulate on top (start=False).
                        for cc in ccs:
                            taug = work.tile([B, GW], BF16, tag="taug", name="taug")
                            nc.vector.tensor_scalar(
                                taug, wrep[:, cc * GW : (cc + 1) * GW],
                                xcol, None, mybir.AluOpType.mult,
                            )
                            nc.vector.tensor_add(
                                pss[cc], taug, brep[:, cc * GW : (cc + 1) * GW]
                            )
                    for k in range(KC):
                        for cc in ccs:
                            nc.tensor.matmul(
                                pss[cc], lhsT=cur[k],
                                rhs=w_sb[k][:, cc * GW : (cc + 1) * GW],
                                start=(k == 0 and augmode != 1),
                                stop=(augdve and k == KC - 1),
                                skip_group_check=augmode == 1,
                            )
                    if not augdve:
                        for cc in ccs:
                            nc.tensor.matmul(
                                pss[cc], lhsT=xa,
                                rhs=wihb[:, cc * GW : (cc + 1) * GW],
                                start=False, stop=True,
                            )
                    # chunks 6,7 are on the next step's critical path: run
                    # their aug-add + eltwise + transpose first in the half
                    order = ccs if half == 0 else [6, 7, 4, 5]
                    if augmode == 2:
                        for cc in order:
                            nc.vector.tensor_add(
                                pss[cc], pss[cc],
                                aug_sb[:, cc * GW : (cc + 1) * GW],
                            )
                    for cc in order:
                        eltwise_single(cc, pss[cc])

            def emit_step_piped(iv_base, local_t, cur, nxt, pending_in):
                """One LSTM step, software-pipelined across the step boundary.

                pending_in: closures (PE transposes + DVE copies of the
                PREVIOUS step's pair-3 h chunks) to emit after this step's
                first independent matmul block. Returns pending_out for the
                next step (empty when this is the last step of the body).
                """
                xa = xap.tile([2, B], BF16, tag="xa")
                if isinstance(iv_base, int):
                    off = 2 * (iv_base + local_t)
                    nc.sync.dma_start(out=xa, in_=xa_d[off : off + 2, :])
                else:
                    off = (iv_base + local_t) * 2
                    nc.sync.dma_start(out=xa, in_=xa_d[ds(off, 2), :])

                def mm(sl, k, cc, start, stop):
                    if k == "aug":
                        nc.tensor.matmul(
                            sl, lhsT=xa, rhs=wihb[:, cc * GW : (cc + 1) * GW],
                            start=start, stop=stop,
                        )
                    else:
                        nc.tensor.matmul(
                            sl, lhsT=cur[k],
                            rhs=w_sb[k][:, cc * GW : (cc + 1) * GW],
                            start=start, stop=stop,
                        )

                def eltwise(p, ps):
                    """gates [B, 2, 512] per chunk-pair -> hbf [B,2,128]."""
                    ps3 = ps.rearrange("p (c x) -> p c x", c=2)
                    sig = work.tile([B, 2, 384], BF16, tag="sig", name="sig")
                    nc.scalar.activation(
                        sig, ps3[:, :, 0:384],
                        mybir.ActivationFunctionType.Sigmoid,
                    )
                    tg = work.tile([B, 2, 128], BF16, tag="tg", name="tg")
                    nc.scalar.activation(
                        tg, ps3[:, :, 384:512], mybir.ActivationFunctionType.Tanh
                    )
                    c3 = c_sb[:, p * 256 : (p + 1) * 256].rearrange(
                        "p (c x) -> p c x", c=2
                    )
                    t1 = work.tile([B, 2, 128], F32, tag="t1", name="t1")
                    nc.vector.tensor_mul(t1, sig[:, :, 128:256], c3)
                    t2 = work.tile([B, 2, 128], BF16, tag="t2", name="t2")
                    nc.vector.tensor_mul(t2, sig[:, :, 0:128], tg)
                    nc.vector.tensor_add(c3, t1, t2)
                    tanc = work.tile([B, 2, 128], BF16, tag="tanc", name="tanc")
                    nc.scalar.activation(
                        tanc, c3, mybir.ActivationFunctionType.Tanh
                    )
                    hbf = work.tile([B, 2, 128], BF16, tag="hbf", name="hbf")
                    nc.vector.tensor_mul(hbf, sig[:, :, 256:384], tanc)
                    return hbf

                def pe_transpose_pair(p, hbf):
                    """PE-transpose both chunks of pair p into nxt (closures)."""
                    outs = []
                    for half in range(2):
                        def do(h=half):
                            pt = ptp.tile([128, B], BF16, tag="pt", name="pt")
                            nc.tensor.transpose(pt, hbf[:, h, :], ident)
                            nc.vector.tensor_copy(nxt[2 * p + h], pt)
                        outs.append(do)
                    return outs

                # ---- P0: k6,k7 deferred past the pending block ----
                ps0 = psum.tile([B, 2 * GW], F32, tag="gates", name="ps0")
                for half in range(2):
                    sl = ps0[:, half * GW : (half + 1) * GW]
                    for k in range(6):
                        mm(sl, k, half, start=(k == 0), stop=False)
                    mm(sl, "aug", half, start=False, stop=False)
                for fn in pending_in:
                    fn()
                for k in (6, 7):
                    for half in range(2):
                        mm(ps0[:, half * GW : (half + 1) * GW], k, half,
                           start=False, stop=(k == 7 and half == 1))
                hbf0 = eltwise(0, ps0)
                for half in range(2):
                    nc.sync.dma_start_transpose(nxt[half], hbf0[:, half, :])

                # ---- P1: standard order, DMA transposes ----
                ps1 = psum.tile([B, 2 * GW], F32, tag="gates", name="ps1")
                for half in range(2):
                    cc = 2 + half
                    sl = ps1[:, half * GW : (half + 1) * GW]
                    for k in range(6):
                        mm(sl, k, cc, start=(k == 0), stop=False)
                    mm(sl, "aug", cc, start=False, stop=False)
                    for k in (6, 7):
                        mm(sl, k, cc, start=False, stop=(k == 7))
                hbf1 = eltwise(1, ps1)
                for half in range(2):
                    nc.sync.dma_start_transpose(nxt[2 + half], hbf1[:, half, :])

                # ---- P2: PE transposes deferred into P3's MM stream ----
                ps2 = psum.tile([B, 2 * GW], F32, tag="gates", name="ps2")
                for half in range(2):
                    cc = 4 + half
                    sl = ps2[:, half * GW : (half + 1) * GW]
                    for k in range(6):
                        mm(sl, k, cc, start=(k == 0), stop=False)
                    mm(sl, "aug", cc, start=False, stop=False)
                    for k in (6, 7):
                        mm(sl, k, cc, start=False, stop=(k == 7))
                hbf2 = eltwise(2, ps2)
                t2_closures = pe_transpose_pair(2, hbf2)

                # ---- P3: first half interleaves P2's transposes ----
                ps3t = psum.tile([B, 2 * GW], F32, tag="gates", name="ps3")
                sl = ps3t[:, 0:GW]
                for k in range(6):
                    mm(sl, k, 6, start=(k == 0), stop=False)
                mm(sl, "aug", 6, start=False, stop=False)
                for fn in t2_closures:
                    fn()
                for k in (6, 7):
                    mm(sl, k, 6, start=False, stop=False)
                sl = ps3t[:, GW : 2 * GW]
                for k in range(6):
                    mm(sl, k, 7, start=(k == 0), stop=False)
                mm(sl, "aug", 7, start=False, stop=False)
                for k in (6, 7):
                    mm(sl, k, 7, start=False, stop=(k == 7))
                hbf3 = eltwise(3, ps3t)
                return pe_transpose_pair(3, hbf3)

            def step(iv_base, local_t, cur, nxt):
                """One LSTM step. iv_base: ScalarValue or int (loop index of the
                body start); local_t: python int offset within the body."""
                xa = xap.tile([2, B], BF16, tag="xa")
                # inside the For_i body only HWDGE DMAs are usable: the loop
                # reset block emits InstIncSwdgeSem for SWDGE queues, which
                # this walrus cannot encode ("ISA wrong length").
                if isinstance(iv_base, int):
                    off = 2 * (iv_base + local_t)
                    nc.sync.dma_start(out=xa, in_=xa_d[off : off + 2, :])
                else:
                    off = (iv_base + local_t) * 2
                    nc.sync.dma_start(out=xa, in_=xa_d[ds(off, 2), :])

                n_pairs = 3 if os.environ.get("TAIL_SINGLE", "0") == "1" else 4
                for p in range(n_pairs):  # pairs of H-chunks
                    ps = psum.tile([B, 2 * GW], F32, tag="gates", name=f"ps{p}")
                    for half in range(2):
                        cc = 2 * p + half
                        sl = ps[:, half * GW : (half + 1) * GW]
                        # K-order [0..5, aug, 6, 7]: defers the previous
                        # step's latest h-chunks by two MM slots, shrinking
                        # the step-boundary stall. Same PSUM group, so no
                        # tile-switch penalty.
                        for k in range(6):
                            nc.tensor.matmul(
                                sl,
                                lhsT=cur[k],
                                rhs=w_sb[k][:, cc * GW : (cc + 1) * GW],
                                start=(k == 0),
                                stop=False,
                            )
                        nc.tensor.matmul(
                            sl,
                            lhsT=xa,
                            rhs=wihb[:, cc * GW : (cc + 1) * GW],
                            start=False,
                            stop=False,
                        )
                        for k in (6, 7):
                            nc.tensor.matmul(
                                sl,
                                lhsT=cur[k],
                                rhs=w_sb[k][:, cc * GW : (cc + 1) * GW],
                                start=False,
                                stop=(k == KC - 1),
                            )
                    # eltwise; psum layout [i0 f0 o0 g0 i1 f1 o1 g1]
                    ps3 = ps.rearrange("p (c x) -> p c x", c=2)
                    sig = work.tile([B, 2, 384], BF16, tag="sig", name="sig")
                    nc.scalar.activation(
                        sig, ps3[:, :, 0:384], mybir.ActivationFunctionType.Sigmoid
                    )
                    tg = work.tile([B, 2, 128], BF16, tag="tg", name="tg")
                    nc.scalar.activation(
                        tg, ps3[:, :, 384:512], mybir.ActivationFunctionType.Tanh
                    )
                    sig_i = sig[:, :, 0:128]
                    sig_f = sig[:, :, 128:256]
                    sig_o = sig[:, :, 256:384]
                    c3 = c_sb[:, p * 256 : (p + 1) * 256].rearrange(
                        "p (c x) -> p c x", c=2
                    )
                    t1 = work.tile([B, 2, 128], F32, tag="t1", name="t1")
                    nc.vector.tensor_mul(t1, sig_f, c3)
                    t2 = work.tile([B, 2, 128], BF16, tag="t2", name="t2")
                    nc.vector.tensor_mul(t2, sig_i, tg)
                    nc.vector.tensor_add(c3, t1, t2)
                    tanc = work.tile([B, 2, 128], BF16, tag="tanc", name="tanc")
                    nc.scalar.activation(
                        tanc, c3, mybir.ActivationFunctionType.Tanh
                    )
                    hbf = work.tile([B, 2, 128], BF16, tag="hbf", name="hbf")
                    nc.vector.tensor_mul(hbf, sig_o, tanc)
                    for half in range(2):
                        if p >= 2:
                            # last pair is on the next step's critical path:
                            # PE transpose (~0.4us) beats the DMA xbar (~1.3us)
                            pt = ptp.tile([128, B], BF16, tag="pt", name="pt")
                            nc.tensor.transpose(pt, hbf[:, half, :], ident)
                            nc.vector.tensor_copy(nxt[2 * p + half], pt)
                        else:
                            nc.sync.dma_start_transpose(
                                nxt[2 * p + half], hbf[:, half, :]
                            )

                for cc in range(2 * n_pairs, NCH):  # tail chunks, single width
                    ps1 = psum.tile([B, GW], F32, tag="gates", name=f"ps1_{cc}")
                    for k in range(KC):
                        nc.tensor.matmul(
                            ps1, lhsT=cur[k],
                            rhs=w_sb[k][:, cc * GW : (cc + 1) * GW],
                            start=(k == 0), stop=False,
                        )
                    nc.tensor.matmul(
                        ps1, lhsT=xa, rhs=wihb[:, cc * GW : (cc + 1) * GW],
                        start=False, stop=True,
                    )
                    sig1 = work.tile([B, 384], BF16, tag="sig1", name="sig1")
                    nc.scalar.activation(
                        sig1, ps1[:, 0:384], mybir.ActivationFunctionType.Sigmoid
                    )
                    tg1 = work.tile([B, 128], BF16, tag="tg1", name="tg1")
                    nc.scalar.activation(
                        tg1, ps1[:, 384:512], mybir.ActivationFunctionType.Tanh
                    )
                    c1 = c_sb[:, cc * 128 : (cc + 1) * 128]
                    t1s = work.tile([B, 128], F32, tag="t1s", name="t1s")
                    nc.vector.tensor_mul(t1s, sig1[:, 128:256], c1)
                    t2s = work.tile([B, 128], BF16, tag="t2s", name="t2s")
                    nc.vector.tensor_mul(t2s, sig1[:, 0:128], tg1)
                    nc.vector.tensor_add(c1, t1s, t2s)
                    tanc1 = work.tile([B, 128], BF16, tag="tanc1", name="tanc1")
                    nc.scalar.activation(
                        tanc1, c1, mybir.ActivationFunctionType.Tanh
                    )
                    hbf1 = work.tile([B, 128], BF16, tag="hbf1", name="hbf1")
                    nc.vector.tensor_mul(hbf1, sig1[:, 256:384], tanc1)
                    pt1 = ptp.tile([128, B], BF16, tag="pt", name="pt1")
                    nc.tensor.transpose(pt1, hbf1, ident)
                    nc.vector.tensor_copy(nxt[cc], pt1)

            pipe = os.environ.get("PIPE", "0") == "1"

            def emit_body(iv_base, n_steps):
                pending = []
                for j in range(n_steps):
                    cur, nxt = (ht_a, ht_b) if j % 2 == 0 else (ht_b, ht_a)
                    if kouter:
                        emit_step_kouter(iv_base, j, cur, nxt)
                    elif pipe:
                        pending = emit_step_piped(iv_base, j, cur, nxt, pending)
                    else:
                        step(iv_base, j, cur, nxt)
                for fn in pending:  # flush at body boundary
                    fn()

            if t_steps == 0:
                pass
            elif t_steps <= unroll:
                repeat_u = int(os.environ.get("KERNEL_REPEAT", "1"))

                if repeat_u == 1:
                    emit_body(0, t_steps)
                else:
                    with tc.For_i(0, repeat_u, 1):
                        emit_body(0, t_steps)
            else:
                hints = tuple(mybir.ALL_ENGINES) if os.environ.get("HINTS", "0") == "1" else ()
                repeat = int(os.environ.get("KERNEL_REPEAT", "1"))

                def inner_loop():
                    with tc.For_i(0, t_steps, unroll, hint_engines=hints) as iv:
                        emit_body(iv, unroll)

                if repeat == 1:
                    inner_loop()
                else:  # timing amplification only: state re-evolves from t=0 xs
                    with tc.For_i(0, repeat, 1):
                        inner_loop()

            ht_fin = ht_a if t_steps % 2 == 0 else ht_b

            # final FC: out = h_T @ fc_W.T + fc_b
            fc_ps = fcp.tile([B, O], F32, tag="fc", name="fcps", bufs=1)
            nc.tensor.matmul(
                fc_ps, lhsT=ones_t, rhs=fcb_t, start=True, stop=False
            )
            for k in range(KC):
                nc.tensor.matmul(
                    fc_ps,
                    lhsT=ht_fin[k],
                    rhs=fcw[:, k * 128 : (k + 1) * 128],
                    start=False,
                    stop=(k == KC - 1),
                )
            out_sb = work.tile([B, O], F32, tag="out_sb")
            nc.vector.tensor_copy(out_sb, fc_ps)
            nc.gpsimd.dma_start(out=out_d, in_=out_sb)

            if debug_state:
                for k in range(NCH):
                    nc.gpsimd.dma_start(out=ht_dbg_d[k], in_=ht_fin[k])
                nc.gpsimd.dma_start(out=c_dbg_d, in_=c_sb)

    if os.environ.get("DEDUP", "1") == "1":
        _dedup_ldweights(nc)
    if split_waits:  # required for walrus codegen; CoreSim chokes on it
        _split_excess_waits(nc)
    return nc


# ------------------------------------------------------------------ host prep
def _prep_inputs(y_hist, W_ih, W_hh, b_ih, b_hh, fc_W, fc_b, h0, c0):
    f32 = np.float32
    bf16 = ml_dtypes.bfloat16
    # per-chunk gate permutation of the 4H rows: [i_c | f_c | o_c | g_c]
    # reference gate order in rows of W_hh is (i, f, g, o) blocks of H
    perm = np.concatenate(
        [
            g * H + c * 128 + np.arange(128)
            for c in range(NCH)
            for g in (0, 1, 3, 2)
        ]
    )
    wt = np.ascontiguousarray(W_hh[perm, :].T).astype(bf16)          # (H, 4H)
    wihb = np.stack([W_ih[:, 0][perm], (b_ih + b_hh)[perm]]).astype(bf16)
    xa = np.empty((2 * T, B), f32)
    xa[0::2] = y_hist.T                                               # x_t rows
    xa[1::2] = 1.0                                                    # ones rows
    xa = xa.astype(bf16)
    ht0 = np.ascontiguousarray(h0.T).astype(bf16)                     # (H, B)
    fcw = np.ascontiguousarray(fc_W.T).astype(bf16)                  # (H, O)
    # device layout for fcw tile: (128, H) with chunk k at cols [128k:128k+128)
    fcw_tile = fcw.reshape(KC, 128, O).transpose(1, 0, 2).reshape(128, H)
    fca = np.stack([fc_b, np.ones(O, f32)]).astype(bf16)              # rhs, ones
    ident = np.eye(128, dtype=f32).astype(bf16)
    wrep = np.broadcast_to(W_ih[:, 0][perm][None, :], (128, 4 * H)).astype(bf16)
    brep = np.broadcast_to((b_ih + b_hh)[perm][None, :], (128, 4 * H)).astype(bf16)
    return {
        "ident": np.asarray(ident),
        "wt": np.asarray(wt),
        "wihb": np.asarray(wihb),
        "xa": np.asarray(xa),
        "wrep": np.ascontiguousarray(wrep),
        "brep": np.ascontiguousarray(brep),
        "xcol": np.ascontiguousarray(y_hist.astype(f32)),
        "ht0": np.asarray(ht0),
        "c0": c0.astype(f32),
        "fcw": np.asarray(fcw_tile),
        "fca": np.asarray(fca),
    }


_CACHE = {}


def _make_runner(nc):
    """Single-core reusable jitted executor (mirrors bass2jax.run_bass_via_pjrt
    but caches the jitted body so repeated kernel() calls skip retracing)."""
    import jax
    from concourse import bass2jax

    bass2jax.install_neuronx_cc_hook()
    partition_name = nc.partition_id_tensor.name if nc.partition_id_tensor else None
    in_names, out_names, out_avals, zero_outs = [], [], [], []
    for alloc in nc.m.functions[0].allocations:
        if not isinstance(alloc, mybir.MemoryLocationSet):
            continue
        name = alloc.memorylocations[0].name
        if alloc.kind == "ExternalInput":
            if name != partition_name:
                in_names.append(name)
        elif alloc.kind == "ExternalOutput":
            shape = tuple(alloc.tensor_shape)
            dtype = mybir.dt.np(alloc.dtype)
            out_names.append(name)
            out_avals.append(jax.core.ShapedArray(shape, dtype))
            zero_outs.append(np.zeros(shape, dtype))
    all_in = list(in_names) + list(out_names)
    if partition_name is not None:
        all_in.append(partition_name)

    def _body(*args):
        operands = list(args)
        if partition_name is not None:
            operands.append(bass2jax.partition_id_tensor())
        return tuple(
            bass2jax._bass_exec_p.bind(
                *operands,
                out_avals=tuple(out_avals),
                in_names=tuple(all_in),
                out_names=tuple(out_names),
                lowering_input_output_aliases=(),
                sim_require_finite=True,
                sim_require_nnan=True,
                nc=nc,
            )
        )

    f = jax.jit(_body, keep_unused=True)
    return f, in_names, out_names, zero_outs


def kernel(y_hist, W_ih, W_hh, b_ih, b_hh, fc_W, fc_b, h0, c0, **kw):
    dev_in = _prep_inputs(
        np.asarray(y_hist, np.float32),
        np.asarray(W_ih, np.float32),
        np.asarray(W_hh, np.float32),
        np.asarray(b_ih, np.float32),
        np.asarray(b_hh, np.float32),
        np.asarray(fc_W, np.float32),
        np.asarray(fc_b, np.float32),
        np.asarray(h0, np.float32),
        np.asarray(c0, np.float32),
    )
    if _N_CORES != 1:
        if "nc" not in _CACHE:
            _CACHE["nc"] = build_program()
        res = run_bass_kernel_spmd(
            _CACHE["nc"],
            [dict(dev_in) for _ in range(_N_CORES)],
            core_ids=list(range(_N_CORES)),
        )
        return np.asarray(res.results[0]["out"], np.float32)
    if "runner" not in _CACHE:
        nc = build_program()
        _CACHE["runner"] = _make_runner(nc)
    f, in_names, out_names, zero_outs = _CACHE["runner"]
    args = [np.asarray(dev_in[n]) for n in in_names] + zero_outs
    outs = f(*args)
    res = {n: np.asarray(outs[i]) for i, n in enumerate(out_names)}
    return np.asarray(res["out"], np.float32)



# revision 27
# speedup vs baseline: 1.0932x; 1.0353x over previous
"""Trainium2 Bass kernel for nn_Decoder (LSTM, B=128 T=512 H=1024 O=128).

Strategy: the T=512 recurrence is inherently sequential and one step's
recurrent matmul (h @ W_hh.T: 128x1024x4096) already saturates a single
NeuronCore's PE for ~9.5us, while any cross-core exchange of h costs an
8-core AllGather floor of ~5us + HBM bounces per step. Tensor-parallel
sharding therefore cannot beat replication, so every core runs the full
recurrence (weights and state replicated); the output is taken from core 0.

Per step (on each core), k-outer schedule (KOUTER=1 default):
  gates = [hT;x_t;1].T @ [W_hh.T; w_ih; b]   in bf16 on the PE,
          accumulated fp32 in PSUM. The 4096 gate columns are split in two
          halves of 4 single-bank [B,512] psums; within a half each
          stationary (h chunk k / xa) is loaded once and streams all 4
          chunk columns (k-outer), and _dedup_ldweights removes the
          redundant ldweights the legalizer would re-emit per matmul.
  Gate columns are host-permuted per 128-wide H-chunk as [i|f|o|g] so one
  strided sigmoid covers i,f,o of a chunk and one tanh covers g.
  c (fp32) and h (bf16) updated on DVE; tanh/sigmoid on ACT; h chunks
  transposed back to lhsT layout [H,B] via the DMA xbar, with chunks 6,7
  (needed earliest next step) processed first in the second half.
"""

import os
import sys

sys.path.insert(0, "/opt/trn_rl_repo")
os.environ.setdefault("JAX_PLATFORMS", "")

from contextlib import ExitStack

import numpy as np
import ml_dtypes

import concourse.bass as bass
import concourse.mybir as mybir
import concourse.tile as tile
from concourse.bass import ds
from concourse.bass_utils import run_bass_kernel_spmd

B, T, H, O = 128, 512, 1024, 128
KC = H // 128          # 8 K-tiles of the contraction over H
NCH = H // 128         # 8 H-chunks of 128 hidden units
GW = 512               # gate columns per H-chunk: [i|f|o|g] x 128
BF16 = mybir.dt.bfloat16
F32 = mybir.dt.float32

_N_CORES = int(os.environ.get("KERNEL_N_CORES", "1"))
# steps per For_i body (must be even: hT ping-pong)
_UNROLL = int(os.environ.get("KERNEL_UNROLL", "4"))


# ---------------------------------------------------------------- drain patch
# walrus codegen limit: InstDrain on the SP engine accepts a single sync-wait
# command, but TileContext's exit drain aggregates one wait per outstanding
# logical processor onto one drain. Split them across a chain of drains.
def _apply_drain_patch():
    import concourse.tile as _tile
    from concourse.vector_clock import ScopedClock as _ScopedClock

    if getattr(_tile.TileContext, "_drain_patch_applied", False):
        return

    def _patched(self, tick_clock, wait_clock):
        drain_inst = self.nc.sync.drain()
        wait_clock.add_sem_waits(
            drain_inst.ins, _ScopedClock({None: tick_clock.global_clock})
        )
        si = drain_inst.ins.sync_info
        waits = list(si.on_wait) if si is not None and si.on_wait else []
        if len(waits) > 1:
            si.on_wait = waits[:1]
            for w in waits[1:]:
                extra = self.nc.sync.drain()
                extra.ins.sync_info = mybir.SyncInfo(on_wait=[w], on_update=[])
        self.nc.all_engine_barrier()
        assert self.sems is not None
        popped = self.nc._tile_sem_poison_stack.pop()
        assert popped is self._sem_poison
        self.nc.clear_and_free_semaphores(list(self.sems.allocated().values()))
        self.nc.all_engine_barrier()

    _tile.TileContext._drain_and_barrier = _patched
    _tile.TileContext._drain_patch_applied = True


# ----------------------------------------------------- wait-splitting post-pass
# This walrus build accepts at most 2 sync-wait commands on ordinary engine
# instructions and only 1 on SP/TPB_CTRL-class instructions (Drain, SP DMA
# triggers). Tile attaches up to ~4. Split the excess onto InstNoOp carriers
# inserted immediately before the offending instruction on the same engine.
_SP_LIKE = ("SP",)


def _wait_limit(inst):
    # empirically: TPB_CTRL (Drain) and S3S3D3_TT (TensorTensor) templates
    # accept a single sync-wait; play safe and allow one everywhere.
    return 1


def _split_excess_waits(nc):
    n_added = 0
    for f in nc.m.functions:
        for bb in f.blocks:
            insts = bb.instructions
            out = []
            changed = False
            for inst in insts:
                si = inst.sync_info
                waits = list(si.on_wait) if si is not None and si.on_wait else []
                lim = _wait_limit(inst)
                if len(waits) > lim:
                    keep = waits[len(waits) - lim :]
                    rest = waits[: len(waits) - lim]
                    nop_lim = 1
                    while rest:
                        chunk, rest = rest[:nop_lim], rest[nop_lim:]
                        nop = mybir.InstNoOp(
                            name=f"waitnop-{n_added}", ins=[], outs=[]
                        )
                        nop.engine = inst.engine
                        nop.sync_info = mybir.SyncInfo(on_wait=chunk, on_update=[])
                        out.append(nop)
                        n_added += 1
                    si.on_wait = keep
                    changed = True
                out.append(inst)
            if changed:
                bb.instructions = out
    return n_added


# ------------------------------------------------- ldweights dedup post-pass
# The IR legalization inserts one InstLdweights per InstMatmult, even when
# consecutive matmuls use the identical stationary AP. The PE array retains
# loaded weights across matmuls, so a reload of the exact same AP (with no
# intervening PE weight change and no write to that tensor) is redundant.
# Removing it saves ~50ns of serial PE time per load on HW.
def _dedup_ldweights(nc):
    n_removed = 0
    for f in nc.m.functions:
        for bb in f.blocks:
            out = []
            last_key = None
            last_tensor = None
            for inst in bb.instructions:
                tn = type(inst).__name__
                if tn == "InstLdweights":
                    key = str(inst.ins[0])
                    if key == last_key:
                        # redundant: drop, but carry sync info onto the
                        # next PE instruction (its matmult follows).
                        si = inst.sync_info
                        if si is not None and (si.on_wait or si.on_update):
                            carry = si
                        else:
                            carry = None
                        n_removed += 1
                        if carry is not None:
                            out.append(("carry", carry))
                        continue
                    last_key = key
                    last_tensor = _ap_tensor_name(inst.ins[0])
                else:
                    # any write to the currently-loaded tensor invalidates
                    if last_tensor is not None:
                        for o in inst.outs:
                            if _ap_tensor_name(o) == last_tensor:
                                last_key = None
                                last_tensor = None
                                break
                out.append(inst)
            # merge carried sync infos onto the next same-engine instruction
            merged = []
            pending = []
            for item in out:
                if isinstance(item, tuple):
                    pending.append(item[1])
                    continue
                if pending and getattr(item, "engine", None) == mybir.EngineType.PE:
                    si = item.sync_info
                    waits = list(si.on_wait) if si is not None and si.on_wait else []
                    updates = (
                        list(si.on_update) if si is not None and si.on_update else []
                    )
                    for c in pending:
                        if c.on_wait:
                            waits.extend(c.on_wait)
                        if c.on_update:
                            updates.extend(c.on_update)
                    item.sync_info = mybir.SyncInfo(on_wait=waits, on_update=updates)
                    pending = []
                merged.append(item)
            assert not pending
            bb.instructions = merged
    return n_removed


def _ap_tensor_name(arg):
    try:
        return arg.memory_location().name
    except Exception:
        try:
            return arg.tensor_name
        except Exception:
            return None


# ------------------------------------------------------------- program build
def build_program(t_steps=T, unroll=_UNROLL, debug_state=False, split_waits=True):
    _apply_drain_patch()
    assert t_steps % unroll == 0 and unroll % 2 == 0
    nc = bass.Bass("TRN2", debug=False)

    wt_d = nc.dram_tensor("wt", (H, 4 * H), BF16, kind="ExternalInput").ap()
    wihb_d = nc.dram_tensor("wihb", (2, 4 * H), BF16, kind="ExternalInput").ap()
    xa_d = nc.dram_tensor("xa", (2 * T, B), BF16, kind="ExternalInput").ap()
    wrep_d = nc.dram_tensor("wrep", (128, 4 * H), BF16, kind="ExternalInput").ap()
    brep_d = nc.dram_tensor("brep", (128, 4 * H), BF16, kind="ExternalInput").ap()
    xcol_d = nc.dram_tensor("xcol", (B, T), F32, kind="ExternalInput").ap()
    ht0_d = nc.dram_tensor("ht0", (H, B), BF16, kind="ExternalInput").ap()
    c0_d = nc.dram_tensor("c0", (B, H), F32, kind="ExternalInput").ap()
    fcw_d = nc.dram_tensor("fcw", (128, H), BF16, kind="ExternalInput").ap()
    fca_d = nc.dram_tensor("fca", (2, 128), BF16, kind="ExternalInput").ap()
    id_d = nc.dram_tensor("ident", (128, 128), BF16, kind="ExternalInput").ap()
    out_d = nc.dram_tensor("out", (B, O), F32, kind="ExternalOutput").ap()
    if debug_state:
        ht_dbg_d = nc.dram_tensor(
            "ht_dbg", (NCH, 128, B), BF16, kind="ExternalOutput"
        ).ap()
        c_dbg_d = nc.dram_tensor("c_dbg", (B, H), F32, kind="ExternalOutput").ap()

    with tile.TileContext(nc) as tc:
        with ExitStack() as ctx:
            consts = ctx.enter_context(tc.tile_pool(name="consts", bufs=1))
            state = ctx.enter_context(tc.tile_pool(name="state", bufs=1))
            work = ctx.enter_context(tc.tile_pool(name="work", bufs=int(os.environ.get("WORK_BUFS", "3"))))
            xap = ctx.enter_context(tc.tile_pool(name="xap", bufs=int(os.environ.get("XA_BUFS", "4"))))
            kouter = os.environ.get("KOUTER", "1") == "1"
            augmode = int(os.environ.get("AUGDVE", "0")) if kouter else 0
            augdve = augmode > 0
            if kouter:
                # 7 single-bank gate psums + 1 FC bank fill all of PSUM; the
                # final FC psum is drawn from the same pool after the loop.
                psum = ctx.enter_context(
                    tc.tile_pool(name="psum", bufs=7, space="PSUM")
                )
                fcp = psum
                ptp = None
            else:
                psum = ctx.enter_context(
                    tc.tile_pool(name="psum", bufs=3, space="PSUM")
                )
                fcp = ctx.enter_context(tc.tile_pool(name="fcp", bufs=1, space="PSUM"))
                ptp = ctx.enter_context(tc.tile_pool(name="ptp", bufs=1, space="PSUM"))


            # resident weights
            w_sb = []
            for k in range(KC):
                w_k = consts.tile([128, 4 * H], BF16, tag=f"w{k}", name=f"w{k}")
                nc.gpsimd.dma_start(out=w_k, in_=wt_d[k * 128 : (k + 1) * 128, :])
                w_sb.append(w_k)
            wihb = consts.tile([2, 4 * H], BF16, tag="wihb")
            nc.gpsimd.dma_start(out=wihb, in_=wihb_d)
            if augdve:
                wrep = consts.tile([128, 4 * H], BF16, tag="wrep")
                nc.gpsimd.dma_start(out=wrep, in_=wrep_d)
                brep = consts.tile([128, 4 * H], BF16, tag="brep")
                nc.gpsimd.dma_start(out=brep, in_=brep_d)
                xcp = ctx.enter_context(tc.tile_pool(name="xcp", bufs=4))
            fcw = consts.tile([128, H], BF16, tag="fcw")
            nc.gpsimd.dma_start(out=fcw, in_=fcw_d)
            ident = consts.tile([128, 128], BF16, tag="ident")
            nc.gpsimd.dma_start(out=ident, in_=id_d)
            fcb_t = consts.tile([1, 128], BF16, tag="fcb_t")
            nc.gpsimd.dma_start(out=fcb_t, in_=fca_d[0:1, :])
            ones_t = consts.tile([1, 128], BF16, tag="ones_t")
            nc.gpsimd.dma_start(out=ones_t, in_=fca_d[1:2, :])

            # state: hT ping-pong chunk tiles, fp32 cell
            ht_a = [state.tile([128, B], BF16, tag=f"hta{k}", name=f"hta{k}") for k in range(NCH)]
            ht_b = [state.tile([128, B], BF16, tag=f"htb{k}", name=f"htb{k}") for k in range(NCH)]
            c_sb = state.tile([B, H], F32, tag="c")
            for k in range(NCH):
                nc.gpsimd.dma_start(
                    out=ht_a[k], in_=ht0_d[k * 128 : (k + 1) * 128, :]
                )
            nc.gpsimd.dma_start(out=c_sb, in_=c0_d)

            def emit_step_kouter(iv_base, local_t, cur, nxt):
                """One LSTM step, k-outer: per half (4 gate-chunks), each
                stationary (h chunk / xa) is loaded once and streams all 4
                chunk columns; _dedup_ldweights removes the redundant
                reloads. Gate psums use 8 single-bank tiles; all h
                transposes go through the DMA xbar."""
                if not augdve:
                    xa = xap.tile([2, B], BF16, tag="xa")
                    if isinstance(iv_base, int):
                        off = 2 * (iv_base + local_t)
                        nc.sync.dma_start(out=xa, in_=xa_d[off : off + 2, :])
                    else:
                        off = (iv_base + local_t) * 2
                        nc.sync.dma_start(out=xa, in_=xa_d[ds(off, 2), :])

                def eltwise_single(cc, ps1):
                    sig1 = work.tile([B, 384], BF16, tag="sig1", name="sig1")
                    nc.scalar.activation(
                        sig1, ps1[:, 0:384], mybir.ActivationFunctionType.Sigmoid
                    )
                    tg1 = work.tile([B, 128], BF16, tag="tg1", name="tg1")
                    nc.scalar.activation(
                        tg1, ps1[:, 384:512], mybir.ActivationFunctionType.Tanh
                    )
                    c1 = c_sb[:, cc * 128 : (cc + 1) * 128]
                    t1s = work.tile([B, 128], F32, tag="t1s", name="t1s")
                    nc.vector.tensor_mul(t1s, sig1[:, 128:256], c1)
                    t2s = work.tile([B, 128], BF16, tag="t2s", name="t2s")
                    nc.vector.tensor_mul(t2s, sig1[:, 0:128], tg1)
                    nc.vector.tensor_add(c1, t1s, t2s)
                    tanc1 = work.tile([B, 128], BF16, tag="tanc1", name="tanc1")
                    nc.scalar.activation(
                        tanc1, c1, mybir.ActivationFunctionType.Tanh
                    )
                    hbf1 = work.tile([B, 128], BF16, tag="hbf1", name="hbf1")
                    nc.vector.tensor_mul(hbf1, sig1[:, 256:384], tanc1)
                    nc.sync.dma_start_transpose(nxt[cc], hbf1)

                if augdve:
                    xcol = xcp.tile([B, 1], F32, tag="xcol")
                    if isinstance(iv_base, int):
                        xo = iv_base + local_t
                        nc.sync.dma_start(out=xcol, in_=xcol_d[:, xo : xo + 1])
                    else:
                        nc.sync.dma_start(
                            out=xcol, in_=xcol_d[:, ds((iv_base + local_t) * 1, 1)]
                        )

                if augmode == 2:
                    # aug = x*w_ih + b computed in SBUF off the critical
                    # path; each chunk gets one DVE add after its matmuls.
                    aug_sb = work.tile([B, 4 * H], BF16, tag="aug", name="aug")
                    nc.vector.tensor_scalar(
                        aug_sb, wrep, xcol, None, mybir.AluOpType.mult
                    )
                    nc.vector.tensor_add(aug_sb, aug_sb, brep)

                for half in range(2):
                    ccs = [0, 1, 2, 3] if half == 0 else [4, 5, 6, 7]
                    pss = {
                        cc: psum.tile([B, GW], F32, tag="gates", name=f"ps{cc}")
                        for cc in ccs
                    }
                    if augmode == 1:
                        # pre-write x*w_ih + b into PSUM on the DVE; the k
                        # matmuls then accumulate on top (start=False).
                        for cc in ccs:
                            taug = work.tile([B, GW], BF16, tag="taug", name="taug")
                            nc.vector.tensor_scalar(
                                taug, wrep[:, cc * GW : (cc + 1) * GW],
                                xcol, None, mybir.AluOpType.mult,
                            )
                            nc.vector.tensor_add(
                                pss[cc], taug, brep[:, cc * GW : (cc + 1) * GW]
                            )
                    # half 0 streams aug before k6/k7: the aug block buys the
                    # previous step's chunks 6,7 (h via eltwise + DMA-T) an
                    # extra ~0.9us before their ldweights.
                    aug_mid = not augdve and half == 0
                    for k in range(KC):
                        if aug_mid and k == 6:
                            for cc in ccs:
                                nc.tensor.matmul(
                                    pss[cc], lhsT=xa,
                                    rhs=wihb[:, cc * GW : (cc + 1) * GW],
                                    start=False, stop=False,
                                )
                        for cc in ccs:
                            nc.tensor.matmul(
                                pss[cc], lhsT=cur[k],
                                rhs=w_sb[k][:, cc * GW : (cc + 1) * GW],
                                start=(k == 0 and augmode != 1),
                                stop=(augdve and k == KC - 1)
                                or (aug_mid and k == KC - 1),
                                skip_group_check=augmode == 1,
                            )
                    if not augdve and not aug_mid:
                        for cc in ccs:
                            nc.tensor.matmul(
                                pss[cc], lhsT=xa,
                                rhs=wihb[:, cc * GW : (cc + 1) * GW],
                                start=False, stop=True,
                            )
                    if os.environ.get("SKIP_ELT", "0") == "1":
                        continue  # timing probe: PE matmul stream only
                    # chunks 6,7 are on the next step's critical path: run
                    # their aug-add + eltwise + transpose first in the half
                    order = ccs if half == 0 else [6, 7, 4, 5]
                    if augmode == 2:
                        for cc in order:
                            nc.vector.tensor_add(
                                pss[cc], pss[cc],
                                aug_sb[:, cc * GW : (cc + 1) * GW],
                            )
                    for cc in order:
                        eltwise_single(cc, pss[cc])

            def emit_step_piped(iv_base, local_t, cur, nxt, pending_in):
                """One LSTM step, software-pipelined across the step boundary.

                pending_in: closures (PE transposes + DVE copies of the
                PREVIOUS step's pair-3 h chunks) to emit after this step's
                first independent matmul block. Returns pending_out for the
                next step (empty when this is the last step of the body).
                """
                xa = xap.tile([2, B], BF16, tag="xa")
                if isinstance(iv_base, int):
                    off = 2 * (iv_base + local_t)
                    nc.sync.dma_start(out=xa, in_=xa_d[off : off + 2, :])
                else:
                    off = (iv_base + local_t) * 2
                    nc.sync.dma_start(out=xa, in_=xa_d[ds(off, 2), :])

                def mm(sl, k, cc, start, stop):
                    if k == "aug":
                        nc.tensor.matmul(
                            sl, lhsT=xa, rhs=wihb[:, cc * GW : (cc + 1) * GW],
                            start=start, stop=stop,
                        )
                    else:
                        nc.tensor.matmul(
                            sl, lhsT=cur[k],
                            rhs=w_sb[k][:, cc * GW : (cc + 1) * GW],
                            start=start, stop=stop,
                        )

                def eltwise(p, ps):
                    """gates [B, 2, 512] per chunk-pair -> hbf [B,2,128]."""
                    ps3 = ps.rearrange("p (c x) -> p c x", c=2)
                    sig = work.tile([B, 2, 384], BF16, tag="sig", name="sig")
                    nc.scalar.activation(
                        sig, ps3[:, :, 0:384],
                        mybir.ActivationFunctionType.Sigmoid,
                    )
                    tg = work.tile([B, 2, 128], BF16, tag="tg", name="tg")
                    nc.scalar.activation(
                        tg, ps3[:, :, 384:512], mybir.ActivationFunctionType.Tanh
                    )
                    c3 = c_sb[:, p * 256 : (p + 1) * 256].rearrange(
                        "p (c x) -> p c x", c=2
                    )
                    t1 = work.tile([B, 2, 128], F32, tag="t1", name="t1")
                    nc.vector.tensor_mul(t1, sig[:, :, 128:256], c3)
                    t2 = work.tile([B, 2, 128], BF16, tag="t2", name="t2")
                    nc.vector.tensor_mul(t2, sig[:, :, 0:128], tg)
                    nc.vector.tensor_add(c3, t1, t2)
                    tanc = work.tile([B, 2, 128], BF16, tag="tanc", name="tanc")
                    nc.scalar.activation(
                        tanc, c3, mybir.ActivationFunctionType.Tanh
                    )
                    hbf = work.tile([B, 2, 128], BF16, tag="hbf", name="hbf")
                    nc.vector.tensor_mul(hbf, sig[:, :, 256:384], tanc)
                    return hbf

                def pe_transpose_pair(p, hbf):
                    """PE-transpose both chunks of pair p into nxt (closures)."""
                    outs = []
                    for half in range(2):
                        def do(h=half):
                            pt = ptp.tile([128, B], BF16, tag="pt", name="pt")
                            nc.tensor.transpose(pt, hbf[:, h, :], ident)
                            nc.vector.tensor_copy(nxt[2 * p + h], pt)
                        outs.append(do)
                    return outs

                # ---- P0: k6,k7 deferred past the pending block ----
                ps0 = psum.tile([B, 2 * GW], F32, tag="gates", name="ps0")
                for half in range(2):
                    sl = ps0[:, half * GW : (half + 1) * GW]
                    for k in range(6):
                        mm(sl, k, half, start=(k == 0), stop=False)
                    mm(sl, "aug", half, start=False, stop=False)
                for fn in pending_in:
                    fn()
                for k in (6, 7):
                    for half in range(2):
                        mm(ps0[:, half * GW : (half + 1) * GW], k, half,
                           start=False, stop=(k == 7 and half == 1))
                hbf0 = eltwise(0, ps0)
                for half in range(2):
                    nc.sync.dma_start_transpose(nxt[half], hbf0[:, half, :])

                # ---- P1: standard order, DMA transposes ----
                ps1 = psum.tile([B, 2 * GW], F32, tag="gates", name="ps1")
                for half in range(2):
                    cc = 2 + half
                    sl = ps1[:, half * GW : (half + 1) * GW]
                    for k in range(6):
                        mm(sl, k, cc, start=(k == 0), stop=False)
                    mm(sl, "aug", cc, start=False, stop=False)
                    for k in (6, 7):
                        mm(sl, k, cc, start=False, stop=(k == 7))
                hbf1 = eltwise(1, ps1)
                for half in range(2):
                    nc.sync.dma_start_transpose(nxt[2 + half], hbf1[:, half, :])

                # ---- P2: PE transposes deferred into P3's MM stream ----
                ps2 = psum.tile([B, 2 * GW], F32, tag="gates", name="ps2")
                for half in range(2):
                    cc = 4 + half
                    sl = ps2[:, half * GW : (half + 1) * GW]
                    for k in range(6):
                        mm(sl, k, cc, start=(k == 0), stop=False)
                    mm(sl, "aug", cc, start=False, stop=False)
                    for k in (6, 7):
                        mm(sl, k, cc, start=False, stop=(k == 7))
                hbf2 = eltwise(2, ps2)
                t2_closures = pe_transpose_pair(2, hbf2)

                # ---- P3: first half interleaves P2's transposes ----
                ps3t = psum.tile([B, 2 * GW], F32, tag="gates", name="ps3")
                sl = ps3t[:, 0:GW]
                for k in range(6):
                    mm(sl, k, 6, start=(k == 0), stop=False)
                mm(sl, "aug", 6, start=False, stop=False)
                for fn in t2_closures:
                    fn()
                for k in (6, 7):
                    mm(sl, k, 6, start=False, stop=False)
                sl = ps3t[:, GW : 2 * GW]
                for k in range(6):
                    mm(sl, k, 7, start=(k == 0), stop=False)
                mm(sl, "aug", 7, start=False, stop=False)
                for k in (6, 7):
                    mm(sl, k, 7, start=False, stop=(k == 7))
                hbf3 = eltwise(3, ps3t)
                return pe_transpose_pair(3, hbf3)

            def step(iv_base, local_t, cur, nxt):
                """One LSTM step. iv_base: ScalarValue or int (loop index of the
                body start); local_t: python int offset within the body."""
                xa = xap.tile([2, B], BF16, tag="xa")
                # inside the For_i body only HWDGE DMAs are usable: the loop
                # reset block emits InstIncSwdgeSem for SWDGE queues, which
                # this walrus cannot encode ("ISA wrong length").
                if isinstance(iv_base, int):
                    off = 2 * (iv_base + local_t)
                    nc.sync.dma_start(out=xa, in_=xa_d[off : off + 2, :])
                else:
                    off = (iv_base + local_t) * 2
                    nc.sync.dma_start(out=xa, in_=xa_d[ds(off, 2), :])

                n_pairs = 3 if os.environ.get("TAIL_SINGLE", "0") == "1" else 4
                for p in range(n_pairs):  # pairs of H-chunks
                    ps = psum.tile([B, 2 * GW], F32, tag="gates", name=f"ps{p}")
                    for half in range(2):
                        cc = 2 * p + half
                        sl = ps[:, half * GW : (half + 1) * GW]
                        # K-order [0..5, aug, 6, 7]: defers the previous
                        # step's latest h-chunks by two MM slots, shrinking
                        # the step-boundary stall. Same PSUM group, so no
                        # tile-switch penalty.
                        for k in range(6):
                            nc.tensor.matmul(
                                sl,
                                lhsT=cur[k],
                                rhs=w_sb[k][:, cc * GW : (cc + 1) * GW],
                                start=(k == 0),
                                stop=False,
                            )
                        nc.tensor.matmul(
                            sl,
                            lhsT=xa,
                            rhs=wihb[:, cc * GW : (cc + 1) * GW],
                            start=False,
                            stop=False,
                        )
                        for k in (6, 7):
                            nc.tensor.matmul(
                                sl,
                                lhsT=cur[k],
                                rhs=w_sb[k][:, cc * GW : (cc + 1) * GW],
                                start=False,
                                stop=(k == KC - 1),
                            )
                    # eltwise; psum layout [i0 f0 o0 g0 i1 f1 o1 g1]
                    ps3 = ps.rearrange("p (c x) -> p c x", c=2)
                    sig = work.tile([B, 2, 384], BF16, tag="sig", name="sig")
                    nc.scalar.activation(
                        sig, ps3[:, :, 0:384], mybir.ActivationFunctionType.Sigmoid
                    )
                    tg = work.tile([B, 2, 128], BF16, tag="tg", name="tg")
                    nc.scalar.activation(
                        tg, ps3[:, :, 384:512], mybir.ActivationFunctionType.Tanh
                    )
                    sig_i = sig[:, :, 0:128]
                    sig_f = sig[:, :, 128:256]
                    sig_o = sig[:, :, 256:384]
                    c3 = c_sb[:, p * 256 : (p + 1) * 256].rearrange(
                        "p (c x) -> p c x", c=2
                    )
                    t1 = work.tile([B, 2, 128], F32, tag="t1", name="t1")
                    nc.vector.tensor_mul(t1, sig_f, c3)
                    t2 = work.tile([B, 2, 128], BF16, tag="t2", name="t2")
                    nc.vector.tensor_mul(t2, sig_i, tg)
                    nc.vector.tensor_add(c3, t1, t2)
                    tanc = work.tile([B, 2, 128], BF16, tag="tanc", name="tanc")
                    nc.scalar.activation(
                        tanc, c3, mybir.ActivationFunctionType.Tanh
                    )
                    hbf = work.tile([B, 2, 128], BF16, tag="hbf", name="hbf")
                    nc.vector.tensor_mul(hbf, sig_o, tanc)
                    for half in range(2):
                        if p >= 2:
                            # last pair is on the next step's critical path:
                            # PE transpose (~0.4us) beats the DMA xbar (~1.3us)
                            pt = ptp.tile([128, B], BF16, tag="pt", name="pt")
                            nc.tensor.transpose(pt, hbf[:, half, :], ident)
                            nc.vector.tensor_copy(nxt[2 * p + half], pt)
                        else:
                            nc.sync.dma_start_transpose(
                                nxt[2 * p + half], hbf[:, half, :]
                            )

                for cc in range(2 * n_pairs, NCH):  # tail chunks, single width
                    ps1 = psum.tile([B, GW], F32, tag="gates", name=f"ps1_{cc}")
                    for k in range(KC):
                        nc.tensor.matmul(
                            ps1, lhsT=cur[k],
                            rhs=w_sb[k][:, cc * GW : (cc + 1) * GW],
                            start=(k == 0), stop=False,
                        )
                    nc.tensor.matmul(
                        ps1, lhsT=xa, rhs=wihb[:, cc * GW : (cc + 1) * GW],
                        start=False, stop=True,
                    )
                    sig1 = work.tile([B, 384], BF16, tag="sig1", name="sig1")
                    nc.scalar.activation(
                        sig1, ps1[:, 0:384], mybir.ActivationFunctionType.Sigmoid
                    )
                    tg1 = work.tile([B, 128], BF16, tag="tg1", name="tg1")
                    nc.scalar.activation(
                        tg1, ps1[:, 384:512], mybir.ActivationFunctionType.Tanh
                    )
                    c1 = c_sb[:, cc * 128 : (cc + 1) * 128]
                    t1s = work.tile([B, 128], F32, tag="t1s", name="t1s")
                    nc.vector.tensor_mul(t1s, sig1[:, 128:256], c1)
                    t2s = work.tile([B, 128], BF16, tag="t2s", name="t2s")
                    nc.vector.tensor_mul(t2s, sig1[:, 0:128], tg1)
                    nc.vector.tensor_add(c1, t1s, t2s)
                    tanc1 = work.tile([B, 128], BF16, tag="tanc1", name="tanc1")
                    nc.scalar.activation(
                        tanc1, c1, mybir.ActivationFunctionType.Tanh
                    )
                    hbf1 = work.tile([B, 128], BF16, tag="hbf1", name="hbf1")
                    nc.vector.tensor_mul(hbf1, sig1[:, 256:384], tanc1)
                    pt1 = ptp.tile([128, B], BF16, tag="pt", name="pt1")
                    nc.tensor.transpose(pt1, hbf1, ident)
                    nc.vector.tensor_copy(nxt[cc], pt1)

            pipe = os.environ.get("PIPE", "0") == "1"

            def emit_body(iv_base, n_steps):
                pending = []
                for j in range(n_steps):
                    cur, nxt = (ht_a, ht_b) if j % 2 == 0 else (ht_b, ht_a)
                    if kouter:
                        emit_step_kouter(iv_base, j, cur, nxt)
                    elif pipe:
                        pending = emit_step_piped(iv_base, j, cur, nxt, pending)
                    else:
                        step(iv_base, j, cur, nxt)
                for fn in pending:  # flush at body boundary
                    fn()

            if t_steps == 0:
                pass
            elif t_steps <= unroll:
                repeat_u = int(os.environ.get("KERNEL_REPEAT", "1"))

                if repeat_u == 1:
                    emit_body(0, t_steps)
                else:
                    with tc.For_i(0, repeat_u, 1):
                        emit_body(0, t_steps)
            else:
                hints = tuple(mybir.ALL_ENGINES) if os.environ.get("HINTS", "0") == "1" else ()
                repeat = int(os.environ.get("KERNEL_REPEAT", "1"))

                def inner_loop():
                    with tc.For_i(0, t_steps, unroll, hint_engines=hints) as iv:
                        emit_body(iv, unroll)

                if repeat == 1:
                    inner_loop()
                else:  # timing amplification only: state re-evolves from t=0 xs
                    with tc.For_i(0, repeat, 1):
                        inner_loop()

            ht_fin = ht_a if t_steps % 2 == 0 else ht_b

            # final FC: out = h_T @ fc_W.T + fc_b
            fc_ps = fcp.tile([B, O], F32, tag="fc", name="fcps", bufs=1)
            nc.tensor.matmul(
                fc_ps, lhsT=ones_t, rhs=fcb_t, start=True, stop=False
            )
            for k in range(KC):
                nc.tensor.matmul(
                    fc_ps,
                    lhsT=ht_fin[k],
                    rhs=fcw[:, k * 128 : (k + 1) * 128],
                    start=False,
                    stop=(k == KC - 1),
                )
            out_sb = work.tile([B, O], F32, tag="out_sb")
            nc.vector.tensor_copy(out_sb, fc_ps)
            nc.gpsimd.dma_start(out=out_d, in_=out_sb)

            if debug_state:
                for k in range(NCH):
                    nc.gpsimd.dma_start(out=ht_dbg_d[k], in_=ht_fin[k])
                nc.gpsimd.dma_start(out=c_dbg_d, in_=c_sb)

    if os.environ.get("DEDUP", "1") == "1":
        _dedup_ldweights(nc)
    if split_waits:  # required for walrus codegen; CoreSim chokes on it
        _split_excess_waits(nc)
    return nc


# ------------------------------------------------------------------ host prep
def _prep_inputs(y_hist, W_ih, W_hh, b_ih, b_hh, fc_W, fc_b, h0, c0):
    f32 = np.float32
    bf16 = ml_dtypes.bfloat16
    # per-chunk gate permutation of the 4H rows: [i_c | f_c | o_c | g_c]
    # reference gate order in rows of W_hh is (i, f, g, o) blocks of H
    perm = np.concatenate(
        [
            g * H + c * 128 + np.arange(128)
            for c in range(NCH)
            for g in (0, 1, 3, 2)
        ]
    )
    wt = np.ascontiguousarray(W_hh[perm, :].T).astype(bf16)          # (H, 4H)
    wihb = np.stack([W_ih[:, 0][perm], (b_ih + b_hh)[perm]]).astype(bf16)
    xa = np.empty((2 * T, B), f32)
    xa[0::2] = y_hist.T                                               # x_t rows
    xa[1::2] = 1.0                                                    # ones rows
    xa = xa.astype(bf16)
    ht0 = np.ascontiguousarray(h0.T).astype(bf16)                     # (H, B)
    fcw = np.ascontiguousarray(fc_W.T).astype(bf16)                  # (H, O)
    # device layout for fcw tile: (128, H) with chunk k at cols [128k:128k+128)
    fcw_tile = fcw.reshape(KC, 128, O).transpose(1, 0, 2).reshape(128, H)
    fca = np.stack([fc_b, np.ones(O, f32)]).astype(bf16)              # rhs, ones
    ident = np.eye(128, dtype=f32).astype(bf16)
    wrep = np.broadcast_to(W_ih[:, 0][perm][None, :], (128, 4 * H)).astype(bf16)
    brep = np.broadcast_to((b_ih + b_hh)[perm][None, :], (128, 4 * H)).astype(bf16)
    return {
        "ident": np.asarray(ident),
        "wt": np.asarray(wt),
        "wihb": np.asarray(wihb),
        "xa": np.asarray(xa),
        "wrep": np.ascontiguousarray(wrep),
        "brep": np.ascontiguousarray(brep),
        "xcol": np.ascontiguousarray(y_hist.astype(f32)),
        "ht0": np.asarray(ht0),
        "c0": c0.astype(f32),
        "fcw": np.asarray(fcw_tile),
        "fca": np.asarray(fca),
    }


_CACHE = {}


def _make_runner(nc):
    """Single-core reusable jitted executor (mirrors bass2jax.run_bass_via_pjrt
    but caches the jitted body so repeated kernel() calls skip retracing)."""
    import jax
    from concourse import bass2jax

    bass2jax.install_neuronx_cc_hook()
    partition_name = nc.partition_id_tensor.name if nc.partition_id_tensor else None
    in_names, out_names, out_avals, zero_outs = [], [], [], []
    for alloc in nc.m.functions[0].allocations:
        if not isinstance(alloc, mybir.MemoryLocationSet):
            continue
        name = alloc.memorylocations[0].name
        if alloc.kind == "ExternalInput":
            if name != partition_name:
                in_names.append(name)
        elif alloc.kind == "ExternalOutput":
            shape = tuple(alloc.tensor_shape)
            dtype = mybir.dt.np(alloc.dtype)
            out_names.append(name)
            out_avals.append(jax.core.ShapedArray(shape, dtype))
            zero_outs.append(np.zeros(shape, dtype))
    all_in = list(in_names) + list(out_names)
    if partition_name is not None:
        all_in.append(partition_name)

    def _body(*args):
        operands = list(args)
        if partition_name is not None:
            operands.append(bass2jax.partition_id_tensor())
        return tuple(
            bass2jax._bass_exec_p.bind(
                *operands,
                out_avals=tuple(out_avals),
                in_names=tuple(all_in),
                out_names=tuple(out_names),
                lowering_input_output_aliases=(),
                sim_require_finite=True,
                sim_require_nnan=True,
                nc=nc,
            )
        )

    f = jax.jit(_body, keep_unused=True)
    return f, in_names, out_names, zero_outs


def kernel(y_hist, W_ih, W_hh, b_ih, b_hh, fc_W, fc_b, h0, c0, **kw):
    dev_in = _prep_inputs(
        np.asarray(y_hist, np.float32),
        np.asarray(W_ih, np.float32),
        np.asarray(W_hh, np.float32),
        np.asarray(b_ih, np.float32),
        np.asarray(b_hh, np.float32),
        np.asarray(fc_W, np.float32),
        np.asarray(fc_b, np.float32),
        np.asarray(h0, np.float32),
        np.asarray(c0, np.float32),
    )
    if _N_CORES != 1:
        if "nc" not in _CACHE:
            _CACHE["nc"] = build_program()
        res = run_bass_kernel_spmd(
            _CACHE["nc"],
            [dict(dev_in) for _ in range(_N_CORES)],
            core_ids=list(range(_N_CORES)),
        )
        return np.asarray(res.results[0]["out"], np.float32)
    if "runner" not in _CACHE:
        nc = build_program()
        _CACHE["runner"] = _make_runner(nc)
    f, in_names, out_names, zero_outs = _CACHE["runner"]
    args = [np.asarray(dev_in[n]) for n in in_names] + zero_outs
    outs = f(*args)
    res = {n: np.asarray(outs[i]) for i, n in enumerate(out_names)}
    return np.asarray(res["out"], np.float32)

